# revision 1
# baseline (speedup 1.0000x reference)
"""Trainium2 Bass kernel for nn_ConduitHydrology (MFD flow accumulation).

The reference graph is the raster 4-neighbor grid on a 1024x1024 raster, so
all segment_sums are 5-point stencil operations. Strategy:
  - Row-partition across 8 cores: core k owns global rows [128k, 128k+128),
    computing on a 192-row slab (32-row halo each side). 32 Jacobi
    iterations x 1-hop stencil => the halo fully absorbs cross-partition
    influence: zero inter-core communication.
  - On-chip layout (interleaved): column = p*8 + c for partition p, chunk
    c in [0,8); rows packed contiguously per chunk (f = c*192 + r for the
    q-domain, c*194 + r for the phi-domain). Row shifts and 7/8 of column
    shifts are free-dim offsets; only the chunk seam (c=7 <-> c=0 of the
    next partition) needs a partition-shift matmul.
  - Per iteration: 8 half-width fp16 products (DVE+GpSimd), 26 fp16
    matmuls on PE accumulating all shifted inflows into fp32 PSUM
    (24 of them with the identity as stationary), and 4 DVE adds
    (fp32 PSUM + fp32 runoff -> fp16 q). The last iteration assembles
    fp32 q for the output math.
The host only pads/slices/relayouts numpy arrays (no arithmetic on host).
"""

import numpy as np

import concourse.bass as bass
import concourse.mybir as mybir
from concourse.bacc import Bacc
from concourse.tile import TileContext
from concourse.bass_utils import run_bass_kernel_spmd

F32 = mybir.dt.float32
F16 = mybir.dt.bfloat16
I32 = mybir.dt.int32
ALU = mybir.AluOpType
ACTF = mybir.ActivationFunctionType

ROWS = COLS = 1024
N_CORES = 8
N_ITERS = 32
P = 128
NCH = 8
RQ = 192          # q-domain rows per slab
RS = 194          # phi-domain rows per slab
FQ = NCH * RQ     # 1536
FS = NCH * RS     # 1552
OWN = 128
OWN0 = 32

RHO_W, GRAV, SEC_PER_A = 1000.0, 9.81, 31556926.0
FLOW_COEFF = 0.0405
PAD_BED = 1.0e30


def build(n_iters=N_ITERS):
    nc = Bacc(None)

    bed_d = nc.declare_dram_parameter("bed", [P, FS], F32, isOutput=False)
    press_d = nc.declare_dram_parameter("press", [P, FS], F32, isOutput=False)
    status_d = nc.declare_dram_parameter("status", [P, FS], I32, isOutput=False)
    melt_d = nc.declare_dram_parameter("melt", [P, FQ], F32, isOutput=False)
    area_d = nc.declare_dram_parameter("area", [P, FQ], F32, isOutput=False)
    cond_d = nc.declare_dram_parameter("conduit", [P, 1024], F32, isOutput=False)
    mats_d = nc.declare_dram_parameter("mats", [P, 896], F32, isOutput=False)
    grad_d = nc.declare_dram_parameter("grad", [P, 1024], F32, isOutput=True)

    # phi-domain / q-domain chunk slices (1D)
    sch = lambda t, c, b, n: t[:, c * RS + b : c * RS + b + n]
    qch = lambda t, c, b, n: t[:, c * RQ + b : c * RQ + b + n]
    # 2D chunked views
    vs = lambda t, b, n: t.rearrange("p (c r) -> p c r", c=NCH)[:, :, b : b + n]
    vq = vs

    # iteration PSUM layout: chunk c at f = 512*(c//2) + 192*(c%2)
    pcf = lambda c: 512 * (c // 2) + 192 * (c % 2)
    # setup PSUM layout: chunk c at f = 256*c
    scf = lambda c: 256 * c

    with TileContext(nc) as tc:
        with (
            tc.tile_pool(name="main", bufs=1) as pool,
            tc.tile_pool(name="ps", bufs=2, space="PSUM") as pspool,
        ):
            def tmp(tag):
                return pool.tile([P, FS], F32, tag=tag, name=tag)

            def psum():
                return pspool.tile([P, 2048], F32, tag="ps", name="ps")

            def emit_group(ops):
                """ops: (out_ap, lhsT, rhs_ap, bank). start=True on the first
                matmul touching each PSUM bank (must cover the bank's used
                region), stop on the last."""
                last = {}
                for i, (o, w, rh, bank) in enumerate(ops):
                    last[bank] = i
                seen = set()
                for i, (o, w, rh, bank) in enumerate(ops):
                    st = bank not in seen
                    seen.add(bank)
                    nc.tensor.matmul(o, w, rh, start=st, stop=(last[bank] == i))

            # ---- constants
            mats = pool.tile([P, 896], F32)
            nc.sync.dma_start(out=mats[:], in_=mats_d[:])
            ID = mats[:, 0:128]
            SHD = mats[:, 128:256]   # out[m] = rhs[m-1]
            SHU = mats[:, 256:384]   # out[m] = rhs[m+1]
            EUP = mats[:, 512:640]   # out[127] = rhs[0]
            FIXC = mats[:, 640:896]  # row 0 = 1e33
            mats16 = pool.tile([P, 384], F16)
            nc.vector.tensor_copy(out=mats16[:], in_=mats[:, 0:384])
            ID16 = mats16[:, 0:128]
            SHD16 = mats16[:, 128:256]
            SHU16 = mats16[:, 256:384]

            # ---- inputs
            bed = tmp("t0")
            press = tmp("t1")
            status = pool.tile([P, FS], I32, tag="t2", name="t2")
            melt = tmp("t3")
            area = tmp("t4")
            cond = pool.tile([P, 1024], F32)
            for t, d, n in ((bed, bed_d, FS), (press, press_d, FS),
                            (status, status_d, FS), (melt, melt_d, FQ),
                            (area, area_d, FQ), (cond, cond_d, 1024)):
                nc.sync.dma_start(out=t[:, 0:n], in_=d[:])

            # ---- runoff (q-domain, fp32)
            r = pool.tile([P, FQ], F32)
            nc.vector.scalar_tensor_tensor(
                out=r[:], in0=melt[:, 0:FQ], scalar=1.0 / SEC_PER_A,
                in1=area[:, 0:FQ], op0=ALU.mult, op1=ALU.mult)

            # ---- potential and core mask (phi-domain)
            phi = tmp("t5")
            nc.vector.scalar_tensor_tensor(
                out=phi[:], in0=bed[:], scalar=RHO_W * GRAV,
                in1=press[:], op0=ALU.mult, op1=ALU.add)
            m = pool.tile([P, FS], F32)
            nc.vector.tensor_scalar(
                out=m[:], in0=status[:], scalar1=0, scalar2=None,
                op0=ALU.is_equal)

            # ---- E-neighbor phi / mask. E neighbor of (p,c): (p,c+1) for
            #      c<7, (p+1, chunk 0) for c=7 (seam); none at (p127,c7).
            def shift_from_east(dst, src, fix=None):
                ps = psum()
                ops = [(ps[:, scf(c) : scf(c) + RS], ID, sch(src, c + 1, 0, RS),
                        c // 2) for c in range(NCH - 1)]
                ops.append((ps[:, scf(7) : scf(7) + RS], SHU, sch(src, 0, 0, RS), 3))
                if fix is not None:
                    ops.append((ps[:, scf(7) : scf(7) + RS], EUP, fix[:, 0:RS], 3))
                emit_group(ops)
                nc.scalar.copy(vs(dst, 0, RS),
                               ps.rearrange("p (c r) -> p c r", c=8)[:, :, 0:RS])

            phiE = tmp("t3")
            shift_from_east(phiE, phi, fix=FIXC)
            mE = tmp("t4")
            shift_from_east(mE, m)

            # ---- directional drops (phi-domain link grids)
            dphiE = tmp("t0")
            nc.vector.tensor_sub(dphiE[:], phi[:], phiE[:])
            dropE = tmp("t1")    # flow col -> col+1, stored at col
            nc.vector.scalar_tensor_tensor(
                out=dropE[:], in0=dphiE[:], scalar=0.0, in1=m[:],
                op0=ALU.max, op1=ALU.mult)
            tw = tmp("t3")
            nc.vector.tensor_scalar(
                out=tw[:], in0=dphiE[:], scalar1=-1.0, scalar2=0.0,
                op0=ALU.mult, op1=ALU.max)
            dropW = pool.tile([P, FS], F32, tag="t2", name="t2f")
            nc.vector.tensor_mul(dropW[:], tw[:], mE[:])

            dphiS = tmp("t4")    # phi[r] - phi[r+1], link at r (per chunk)
            nc.vector.tensor_sub(vs(dphiS, 0, RS - 1), vs(phi, 0, RS - 1),
                                 vs(phi, 1, RS - 1))
            dropS = tmp("t6")    # flow r -> r+1, stored at r
            nc.vector.scalar_tensor_tensor(
                out=vs(dropS, 0, RS - 1), in0=vs(dphiS, 0, RS - 1), scalar=0.0,
                in1=vs(m, 0, RS - 1), op0=ALU.max, op1=ALU.mult)
            tn = tmp("t3")
            nc.vector.tensor_scalar(
                out=vs(tn, 0, RS - 1), in0=vs(dphiS, 0, RS - 1), scalar1=-1.0,
                scalar2=0.0, op0=ALU.mult, op1=ALU.max)
            dropN = tmp("t7")    # flow r+1 -> r, stored at r
            nc.vector.tensor_mul(vs(dropN, 0, RS - 1), vs(tn, 0, RS - 1),
                                 vs(m, 1, RS - 1))

            # ---- outgoing-W drop at its source (q-domain):
            #      dW[p,c] = dropW[(p,c-1)] | dropW[(p-1, c7)]
            psW = psum()
            ops = [(psW[:, scf(c) : scf(c) + RQ], ID, sch(dropW, c - 1, 1, RQ),
                    c // 2) for c in range(1, NCH)]
            ops.append((psW[:, scf(0) : scf(0) + RQ], SHD, sch(dropW, 7, 1, RQ), 0))
            emit_group(ops)
            dW = pool.tile([P, FQ], F32, tag="t3", name="t3w")
            nc.scalar.copy(vq(dW, 0, RQ),
                           psW.rearrange("p (c r) -> p c r", c=8)[:, :, 0:RQ])

            # ---- total outgoing drop (q-domain)
            psT = psum()
            ops = []
            for c in range(NCH):
                o = psT[:, scf(c) : scf(c) + RQ]
                ops += [(o, ID, sch(dropE, c, 1, RQ), c // 2),
                        (o, ID, sch(dropS, c, 1, RQ), c // 2),
                        (o, ID, sch(dropN, c, 0, RQ), c // 2),
                        (o, ID, qch(dW, c, 0, RQ), c // 2)]
            emit_group(ops)
            tds = pool.tile([P, FQ], F32, tag="t0", name="t0t")
            nc.vector.tensor_scalar(
                out=vq(tds, 0, RQ),
                in0=psT.rearrange("p (c r) -> p c r", c=8)[:, :, 0:RQ],
                scalar1=1.0e-30, scalar2=None, op0=ALU.max)
            recip = pool.tile([P, FQ], F32, tag="t4", name="t4r")
            nc.vector.reciprocal(recip[:], tds[:])

            # ---- outflow fractions, cast to fp16 (q-domain, source node)
            fE = pool.tile([P, FQ], F16)
            fW = pool.tile([P, FQ], F16)
            fS = pool.tile([P, FQ], F16)
            fN = pool.tile([P, FQ], F16)
            nc.vector.tensor_mul(vq(fE, 0, RQ), vs(dropE, 1, RQ), vq(recip, 0, RQ))
            nc.vector.tensor_mul(fW[:], dW[:], recip[:])
            nc.vector.tensor_mul(vq(fS, 0, RQ), vs(dropS, 1, RQ), vq(recip, 0, RQ))
            nc.vector.tensor_mul(vq(fN, 0, RQ), vs(dropN, 0, RQ), vq(recip, 0, RQ))

            # slab-edge outflow rows leave the slab; zero them so the
            # pair-merged row-shift matmuls bleed exact zeros across the
            # chunk boundary inside each PSUM bank.
            nc.vector.memset(vq(fS, RQ - 1, 1), 0.0)
            nc.vector.memset(vq(fN, 0, 1), 0.0)

            # ---- discharge iteration state (two half tensors so the
            # per-bank assembly -> product dependency is tile-granular)
            H2 = FQ // 2
            q16a = pool.tile([P, H2], F16)
            q16b = pool.tile([P, H2], F16)
            nc.scalar.copy(q16a[:], r[:, 0:H2])
            nc.scalar.copy(q16b[:], r[:, H2:FQ])
            q32 = pool.tile([P, FQ], F32)
            oE = pool.tile([P, FQ], F16)
            oW = pool.tile([P, FQ], F16)
            oS = pool.tile([P, FQ], F16)
            oN = pool.tile([P, FQ], F16)

            H = FQ // 2
            for it in range(n_iters):
                lastit = it == n_iters - 1
                qdst = q32
                # products. DVE: oW/oE at pair granularity, ordered so the
                # bank-0 seam operand (oE pair 3) is ready early; GpSimd
                # (slower, ~2.5 cyc/elem floor) gets 3 halves of oS/oN and
                # DVE absorbs the last.
                PR = 384
                def q16s(pr):
                    t = q16a if pr < 2 else q16b
                    lo = (pr % 2) * PR
                    return t[:, lo : lo + PR]
                for pr in (0, 1, 2, 3):
                    sl = slice(pr * PR, (pr + 1) * PR)
                    nc.vector.tensor_mul(oW[:, sl], fW[:, sl], q16s(pr))
                for pr in (3, 0, 1, 2):
                    sl = slice(pr * PR, (pr + 1) * PR)
                    nc.vector.tensor_mul(oE[:, sl], fE[:, sl], q16s(pr))
                nc.gpsimd.tensor_mul(oS[:, 0:H], fS[:, 0:H], q16a[:])
                nc.gpsimd.tensor_mul(oN[:, 0:H], fN[:, 0:H], q16a[:])
                nc.gpsimd.tensor_mul(oS[:, H:FQ], fS[:, H:FQ], q16b[:])
                nc.vector.tensor_mul(oN[:, H:FQ], fN[:, H:FQ], q16b[:])

                ps = psum()
                # Per-bank, in order: starter (covers the bank's whole used
                # region), accumulators, then the q assembly for that bank
                # so DVE drains banks while PE works on later ones.
                bank_ops = [
                    [   # bank 0: chunks 0,1
                        (ps[:, 0:384], ID16, oW[:, 192:576], 0),
                        (ps[:, 192:384], ID16, oE[:, 0:192], 0),
                        (ps[:, 0:192], SHD16, oE[:, 1344:1536], 0),
                        (ps[:, 1:384], ID16, oS[:, 0:383], 0),
                        (ps[:, 0:383], ID16, oN[:, 1:384], 0),
                    ],
                    [   # bank 1: chunks 2,3
                        (ps[:, 512:896], ID16, oW[:, 576:960], 1),
                        (ps[:, 512:896], ID16, oE[:, 192:576], 1),
                        (ps[:, 513:896], ID16, oS[:, 384:767], 1),
                        (ps[:, 512:895], ID16, oN[:, 385:768], 1),
                    ],
                    [   # bank 2: chunks 4,5
                        (ps[:, 1024:1408], ID16, oW[:, 960:1344], 2),
                        (ps[:, 1024:1408], ID16, oE[:, 576:960], 2),
                        (ps[:, 1025:1408], ID16, oS[:, 768:1151], 2),
                        (ps[:, 1024:1407], ID16, oN[:, 769:1152], 2),
                    ],
                    [   # bank 3: chunks 6,7
                        (ps[:, 1536:1920], ID16, oE[:, 960:1344], 3),
                        (ps[:, 1536:1728], ID16, oW[:, 1344:1536], 3),
                        (ps[:, 1728:1920], SHU16, oW[:, 0:192], 3),
                        (ps[:, 1537:1920], ID16, oS[:, 1152:1535], 3),
                        (ps[:, 1536:1919], ID16, oN[:, 1153:1536], 3),
                    ],
                ]
                for b in range(4):
                    for i, (o, w, rh, _bk) in enumerate(bank_ops[b]):
                        nc.tensor.matmul(o, w, rh, start=(i == 0),
                                         stop=(i == len(bank_ops[b]) - 1))
                    if lastit:
                        odst = qdst[:, 384 * b : 384 * b + 384]
                    else:
                        qt = q16a if b < 2 else q16b
                        odst = qt[:, (b % 2) * 384 : (b % 2) * 384 + 384]
                    nc.vector.tensor_add(
                        out=odst,
                        in0=ps[:, 512 * b : 512 * b + 384],
                        in1=r[:, 384 * b : 384 * b + 384])

            # ---- gradient on owned rows (compact [p, c*128+j] layout)
            s1 = pool.tile([P, 1024], F32, tag="f0", name="f0")
            nc.scalar.sqrt(s1[:], cond[:])
            s2 = pool.tile([P, 1024], F32, tag="f1", name="f1")
            nc.scalar.sqrt(s2[:], s1[:])
            c125 = pool.tile([P, 1024], F32, tag="f0", name="f0b")
            nc.vector.tensor_mul(c125[:], cond[:], s2[:])
            k0 = pool.tile([P, 1024], F32, tag="f1", name="f1b")
            nc.scalar.activation(k0[:], c125[:], ACTF.Square,
                                 scale=float(FLOW_COEFF))
            vo = lambda t: t.rearrange("p (c j) -> p c j", c=NCH)
            km = pool.tile([P, 1024], F32, tag="f0", name="f0c")
            nc.vector.tensor_mul(vo(km), vo(k0), vs(m, OWN0 + 1, OWN))
            q2 = pool.tile([P, 1024], F32, tag="f1", name="f1c")
            nc.scalar.activation(vo(q2), vq(q32, OWN0, OWN), ACTF.Square)
            g = pool.tile([P, 1024], F32, tag="f2", name="f2")
            nc.vector.tensor_mul(g[:], q2[:], km[:])

            nc.sync.dma_start(out=grad_d[:], in_=g[:])

    nc.finalize()
    return nc


# ------------------------------------------------------------------ host side

def _mats():
    ident = np.eye(P, dtype=np.float32)
    shd = np.zeros((P, P), np.float32)
    shd[np.arange(P - 1), np.arange(1, P)] = 1.0      # out[m] = rhs[m-1]
    shu = np.zeros((P, P), np.float32)
    shu[np.arange(1, P), np.arange(P - 1)] = 1.0      # out[m] = rhs[m+1]
    edn = np.zeros((P, P), np.float32)
    edn[P - 1, 0] = 1.0
    eup = np.zeros((P, P), np.float32)
    eup[0, P - 1] = 1.0
    fixc = np.zeros((P, 2 * P), np.float32)
    fixc[0, :] = 1.0e33
    return np.concatenate([ident, shd, shu, edn, eup, fixc], axis=1)


def _to_dev(slab):
    """[rows, 1024] row-major slab -> [128, 8*rows], col = p*8 + c."""
    rows = slab.shape[0]
    return np.ascontiguousarray(
        slab.reshape(rows, P, NCH).transpose(1, 2, 0)).reshape(P, NCH * rows)


_BUILT = None


def _get_built():
    global _BUILT
    if _BUILT is None:
        _BUILT = build()
    return _BUILT


def _make_in_maps(melt_rate, bedrock_elevation, water_pressure, cell_area,
                  conduit_size, status_at_node):
    grid = lambda a: np.asarray(a).reshape(ROWS, COLS)
    bed = grid(bedrock_elevation).astype(np.float32)
    press = grid(water_pressure).astype(np.float32)
    status = grid(status_at_node).astype(np.int32)
    melt = grid(melt_rate).astype(np.float32)
    area = grid(cell_area).astype(np.float32)
    cond = grid(conduit_size).astype(np.float32)

    gp = 33
    bedp = np.full((ROWS + 2 * gp, COLS), PAD_BED, np.float32)
    bedp[gp:gp + ROWS] = bed
    pressp = np.zeros((ROWS + 2 * gp, COLS), np.float32)
    pressp[gp:gp + ROWS] = press
    statusp = np.ones((ROWS + 2 * gp, COLS), np.int32)
    statusp[gp:gp + ROWS] = status
    gq = 32
    meltp = np.zeros((ROWS + 2 * gq, COLS), np.float32)
    meltp[gq:gq + ROWS] = melt
    areap = np.zeros((ROWS + 2 * gq, COLS), np.float32)
    areap[gq:gq + ROWS] = area

    mats = _mats()
    in_maps = []
    for k in range(N_CORES):
        r0 = k * OWN
        in_maps.append({
            "bed": _to_dev(bedp[r0 : r0 + RS]),
            "press": _to_dev(pressp[r0 : r0 + RS]),
            "status": _to_dev(statusp[r0 : r0 + RS]),
            "melt": _to_dev(meltp[r0 : r0 + RQ]),
            "area": _to_dev(areap[r0 : r0 + RQ]),
            "conduit": _to_dev(cond[r0 : r0 + OWN]),
            "mats": mats,
        })
    return in_maps


def _from_dev(res_maps):
    out = np.empty((ROWS, COLS), np.float32)
    for k in range(N_CORES):
        g = res_maps[k]["grad"].reshape(P, NCH, OWN)    # [p, c, j]
        out[k * OWN : (k + 1) * OWN] = g.transpose(2, 0, 1).reshape(OWN, COLS)
    return out.ravel()


def run(inputs, trace=False, **kwargs):
    nc = _get_built()
    in_maps = _make_in_maps(
        inputs["melt_rate"], inputs["bedrock_elevation"],
        inputs["water_pressure"], inputs["cell_area"],
        inputs["conduit_size"], inputs["status_at_node"])
    res = run_bass_kernel_spmd(nc, in_maps, list(range(N_CORES)),
                               trace=trace, **kwargs)
    return _from_dev(res.results), res


def kernel(**inputs):
    out, _ = run(inputs)
    return out



# revision 2
# speedup vs baseline: 1.5722x; 1.5722x over previous
"""Trainium2 Bass kernel for nn_ConduitHydrology (MFD flow accumulation).

The reference graph is the raster 4-neighbor grid on a 1024x1024 raster, so
all segment_sums are 5-point stencil operations. Strategy:
  - Row-partition across 8 cores: core k owns global rows [128k, 128k+128),
    computing on a 192-row slab (32-row halo each side). 32 Jacobi
    iterations x 1-hop stencil => the halo fully absorbs cross-partition
    influence: zero inter-core communication.
  - On-chip layout (interleaved): column = p*8 + c for partition p, chunk
    c in [0,8); rows packed contiguously per chunk (f = c*192 + r for the
    q-domain, c*194 + r for the phi-domain). Row shifts and 7/8 of column
    shifts are free-dim offsets; only the chunk seam (c=7 <-> c=0 of the
    next partition) needs a partition-shift matmul.
  - Per iteration: 4 wide DVE products (one per PSUM bank block; fractions
    stored field-major in one contiguous fALL tensor so each product is a
    single field-strided op against a broadcast q slice), ~24 bf16 matmuls
    on PE accumulating all shifted inflows PLUS the runoff into fp32 PSUM,
    and 4 ACT-engine copies draining PSUM -> bf16 q (fp32 on the last
    iteration). GpSimd is kept idle: concurrent GpSimd tensor ops stretch
    DVE ops 3-5x (SBUF contention).
The host only pads/slices/relayouts numpy arrays (no arithmetic on host).
"""

import numpy as np

import concourse.bass as bass
import concourse.mybir as mybir
from concourse.bacc import Bacc
from concourse.tile import TileContext
from concourse.bass_utils import run_bass_kernel_spmd

F32 = mybir.dt.float32
F16 = mybir.dt.bfloat16
I32 = mybir.dt.int32
ALU = mybir.AluOpType
ACTF = mybir.ActivationFunctionType

ROWS = COLS = 1024
N_CORES = 8
N_ITERS = 32
P = 128
NCH = 8
RQ = 192          # q-domain rows per slab
RS = 194          # phi-domain rows per slab
FQ = NCH * RQ     # 1536
FS = NCH * RS     # 1552
OWN = 128
OWN0 = 32

RHO_W, GRAV, SEC_PER_A = 1000.0, 9.81, 31556926.0
FLOW_COEFF = 0.0405
PAD_BED = 1.0e30


def build(n_iters=N_ITERS):
    nc = Bacc(None)

    bed_d = nc.declare_dram_parameter("bed", [P, FS], F32, isOutput=False)
    press_d = nc.declare_dram_parameter("press", [P, FS], F32, isOutput=False)
    status_d = nc.declare_dram_parameter("status", [P, FS], I32, isOutput=False)
    melt_d = nc.declare_dram_parameter("melt", [P, FQ], F32, isOutput=False)
    area_d = nc.declare_dram_parameter("area", [P, FQ], F32, isOutput=False)
    cond_d = nc.declare_dram_parameter("conduit", [P, 1024], F32, isOutput=False)
    mats_d = nc.declare_dram_parameter("mats", [P, 896], F32, isOutput=False)
    grad_d = nc.declare_dram_parameter("grad", [P, 1024], F32, isOutput=True)

    # phi-domain / q-domain chunk slices (1D)
    sch = lambda t, c, b, n: t[:, c * RS + b : c * RS + b + n]
    qch = lambda t, c, b, n: t[:, c * RQ + b : c * RQ + b + n]
    # 2D chunked views
    vs = lambda t, b, n: t.rearrange("p (c r) -> p c r", c=NCH)[:, :, b : b + n]
    vq = vs

    # setup PSUM layout: chunk c at f = 256*c
    scf = lambda c: 256 * c

    with TileContext(nc) as tc:
        with (
            tc.tile_pool(name="main", bufs=1) as pool,
            tc.tile_pool(name="ps", bufs=2, space="PSUM") as pspool,
        ):
            def tmp(tag):
                return pool.tile([P, FS], F32, tag=tag, name=tag)

            def psum():
                return pspool.tile([P, 2048], F32, tag="ps", name="ps")

            def emit_group(ops):
                """ops: (out_ap, lhsT, rhs_ap, bank). start=True on the first
                matmul touching each PSUM bank (must cover the bank's used
                region), stop on the last."""
                last = {}
                for i, (o, w, rh, bank) in enumerate(ops):
                    last[bank] = i
                seen = set()
                for i, (o, w, rh, bank) in enumerate(ops):
                    st = bank not in seen
                    seen.add(bank)
                    nc.tensor.matmul(o, w, rh, start=st, stop=(last[bank] == i))

            # ---- constants
            mats = pool.tile([P, 896], F32)
            nc.sync.dma_start(out=mats[:], in_=mats_d[:])
            ID = mats[:, 0:128]
            SHD = mats[:, 128:256]   # out[m] = rhs[m-1]
            SHU = mats[:, 256:384]   # out[m] = rhs[m+1]
            EUP = mats[:, 512:640]   # out[127] = rhs[0]
            FIXC = mats[:, 640:896]  # row 0 = 1e33
            mats16 = pool.tile([P, 384], F16)
            nc.vector.tensor_copy(out=mats16[:], in_=mats[:, 0:384])
            ID16 = mats16[:, 0:128]
            SHD16 = mats16[:, 128:256]
            SHU16 = mats16[:, 256:384]

            # ---- inputs
            bed = tmp("t0")
            press = tmp("t1")
            status = pool.tile([P, FS], I32, tag="t2", name="t2")
            melt = tmp("t3")
            area = tmp("t4")
            cond = pool.tile([P, 1024], F32)
            for t, d, n in ((bed, bed_d, FS), (press, press_d, FS),
                            (status, status_d, FS), (melt, melt_d, FQ),
                            (area, area_d, FQ), (cond, cond_d, 1024)):
                nc.sync.dma_start(out=t[:, 0:n], in_=d[:])

            # ---- runoff (q-domain, fp32)
            r = pool.tile([P, FQ], F32)
            nc.vector.scalar_tensor_tensor(
                out=r[:], in0=melt[:, 0:FQ], scalar=1.0 / SEC_PER_A,
                in1=area[:, 0:FQ], op0=ALU.mult, op1=ALU.mult)

            # ---- potential and core mask (phi-domain)
            phi = tmp("t5")
            nc.vector.scalar_tensor_tensor(
                out=phi[:], in0=bed[:], scalar=RHO_W * GRAV,
                in1=press[:], op0=ALU.mult, op1=ALU.add)
            m = pool.tile([P, FS], F32)
            nc.vector.tensor_scalar(
                out=m[:], in0=status[:], scalar1=0, scalar2=None,
                op0=ALU.is_equal)

            # ---- E-neighbor phi / mask. E neighbor of (p,c): (p,c+1) for
            #      c<7, (p+1, chunk 0) for c=7 (seam); none at (p127,c7).
            def shift_from_east(dst, src, fix=None):
                ps = psum()
                ops = [(ps[:, scf(c) : scf(c) + RS], ID, sch(src, c + 1, 0, RS),
                        c // 2) for c in range(NCH - 1)]
                ops.append((ps[:, scf(7) : scf(7) + RS], SHU, sch(src, 0, 0, RS), 3))
                if fix is not None:
                    ops.append((ps[:, scf(7) : scf(7) + RS], EUP, fix[:, 0:RS], 3))
                emit_group(ops)
                nc.scalar.copy(vs(dst, 0, RS),
                               ps.rearrange("p (c r) -> p c r", c=8)[:, :, 0:RS])

            phiE = tmp("t3")
            shift_from_east(phiE, phi, fix=FIXC)
            mE = tmp("t4")
            shift_from_east(mE, m)

            # ---- directional drops (phi-domain link grids)
            dphiE = tmp("t0")
            nc.vector.tensor_sub(dphiE[:], phi[:], phiE[:])
            dropE = tmp("t1")    # flow col -> col+1, stored at col
            nc.vector.scalar_tensor_tensor(
                out=dropE[:], in0=dphiE[:], scalar=0.0, in1=m[:],
                op0=ALU.max, op1=ALU.mult)
            tw = tmp("t3")
            nc.vector.tensor_scalar(
                out=tw[:], in0=dphiE[:], scalar1=-1.0, scalar2=0.0,
                op0=ALU.mult, op1=ALU.max)
            dropW = pool.tile([P, FS], F32, tag="t2", name="t2f")
            nc.vector.tensor_mul(dropW[:], tw[:], mE[:])

            dphiS = tmp("t4")    # phi[r] - phi[r+1], link at r (per chunk)
            nc.vector.tensor_sub(vs(dphiS, 0, RS - 1), vs(phi, 0, RS - 1),
                                 vs(phi, 1, RS - 1))
            dropS = tmp("t6")    # flow r -> r+1, stored at r
            nc.vector.scalar_tensor_tensor(
                out=vs(dropS, 0, RS - 1), in0=vs(dphiS, 0, RS - 1), scalar=0.0,
                in1=vs(m, 0, RS - 1), op0=ALU.max, op1=ALU.mult)
            tn = tmp("t3")
            nc.vector.tensor_scalar(
                out=vs(tn, 0, RS - 1), in0=vs(dphiS, 0, RS - 1), scalar1=-1.0,
                scalar2=0.0, op0=ALU.mult, op1=ALU.max)
            dropN = tmp("t7")    # flow r+1 -> r, stored at r
            nc.vector.tensor_mul(vs(dropN, 0, RS - 1), vs(tn, 0, RS - 1),
                                 vs(m, 1, RS - 1))

            # ---- outgoing-W drop at its source (q-domain):
            #      dW[p,c] = dropW[(p,c-1)] | dropW[(p-1, c7)]
            psW = psum()
            ops = [(psW[:, scf(c) : scf(c) + RQ], ID, sch(dropW, c - 1, 1, RQ),
                    c // 2) for c in range(1, NCH)]
            ops.append((psW[:, scf(0) : scf(0) + RQ], SHD, sch(dropW, 7, 1, RQ), 0))
            emit_group(ops)
            dW = pool.tile([P, FQ], F32, tag="t3", name="t3w")
            nc.scalar.copy(vq(dW, 0, RQ),
                           psW.rearrange("p (c r) -> p c r", c=8)[:, :, 0:RQ])

            # ---- total outgoing drop (q-domain)
            psT = psum()
            ops = []
            for c in range(NCH):
                o = psT[:, scf(c) : scf(c) + RQ]
                ops += [(o, ID, sch(dropE, c, 1, RQ), c // 2),
                        (o, ID, sch(dropS, c, 1, RQ), c // 2),
                        (o, ID, sch(dropN, c, 0, RQ), c // 2),
                        (o, ID, qch(dW, c, 0, RQ), c // 2)]
            emit_group(ops)
            tds = pool.tile([P, FQ], F32, tag="t0", name="t0t")
            nc.vector.tensor_scalar(
                out=vq(tds, 0, RQ),
                in0=psT.rearrange("p (c r) -> p c r", c=8)[:, :, 0:RQ],
                scalar1=1.0e-30, scalar2=None, op0=ALU.max)
            recip = pool.tile([P, FQ], F32, tag="t4", name="t4r")
            nc.vector.reciprocal_approx_fast(out=recip[:], in_=tds[:])

            # ---- outflow fractions, field-major in one contiguous bf16
            #      tensor: fALL = [fE | fW | fS | fN], each [P, FQ].
            fALL = pool.tile([P, 4 * FQ], F16)
            fE = fALL[:, 0 * FQ : 1 * FQ]
            fW = fALL[:, 1 * FQ : 2 * FQ]
            fS = fALL[:, 2 * FQ : 3 * FQ]
            fN = fALL[:, 3 * FQ : 4 * FQ]
            vf = lambda t: t.rearrange("p (c r) -> p c r", c=NCH)
            nc.vector.tensor_mul(vf(fE), vs(dropE, 1, RQ), vq(recip, 0, RQ))
            nc.vector.tensor_mul(fW[:], dW[:], recip[:])
            nc.vector.tensor_mul(vf(fS), vs(dropS, 1, RQ), vq(recip, 0, RQ))
            nc.vector.tensor_mul(vf(fN), vs(dropN, 0, RQ), vq(recip, 0, RQ))

            # slab-edge outflow rows leave the slab; zero them so the
            # pair-merged row-shift matmuls bleed exact zeros across the
            # chunk boundary inside each PSUM bank.
            nc.vector.memset(vf(fS)[:, :, RQ - 1 : RQ], 0.0)
            nc.vector.memset(vf(fN)[:, :, 0:1], 0.0)

            # ---- discharge iteration state
            r16 = pool.tile([P, FQ], F16)
            nc.scalar.copy(r16[:], r[:])
            q16 = pool.tile([P, FQ], F16)
            nc.scalar.copy(q16[:], r[:])
            q32 = pool.tile([P, FQ], F32)
            oA = pool.tile([P, 4 * FQ], F16)    # iteration products, ping
            oB = pool.tile([P, 4 * FQ], F16)    # iteration products, pong

            B = 384   # q columns per PSUM bank block
            # field-strided product views: block b covers q cols
            # [384b, 384b+384) across all 4 fields.
            def fblk(t, b):
                return t.rearrange("p (f x) -> p f x", f=4)[:, :, b * B : (b + 1) * B]

            for it in range(n_iters):
                lastit = it == n_iters - 1
                o = oA if it % 2 == 0 else oB
                oE = o[:, 0 * FQ : 1 * FQ]
                oW = o[:, 1 * FQ : 2 * FQ]
                oS = o[:, 2 * FQ : 3 * FQ]
                oN = o[:, 3 * FQ : 4 * FQ]

                # products: one wide DVE op per bank block (4 fields x 384),
                # q slice broadcast across the field dim.
                for b in (1, 0, 2, 3):
                    qb = q16[:, b * B : (b + 1) * B]
                    nc.vector.tensor_mul(
                        fblk(o, b), fblk(fALL, b),
                        qb.unsqueeze(1).broadcast_to([P, 4, B]))

                ps = psum()
                bank_ops = [
                    [   # bank 0: chunks 0,1
                        (ps[:, 0:384], ID16, oW[:, 192:576]),
                        (ps[:, 192:384], ID16, oE[:, 0:192]),
                        (ps[:, 0:192], SHD16, oE[:, 1344:1536]),
                        (ps[:, 1:384], ID16, oS[:, 0:383]),
                        (ps[:, 0:383], ID16, oN[:, 1:384]),
                        (ps[:, 0:384], ID16, r16[:, 0:384]),
                    ],
                    [   # bank 1: chunks 2,3
                        (ps[:, 512:896], ID16, oW[:, 576:960]),
                        (ps[:, 512:896], ID16, oE[:, 192:576]),
                        (ps[:, 513:896], ID16, oS[:, 384:767]),
                        (ps[:, 512:895], ID16, oN[:, 385:768]),
                        (ps[:, 512:896], ID16, r16[:, 384:768]),
                    ],
                    [   # bank 2: chunks 4,5
                        (ps[:, 1024:1408], ID16, oW[:, 960:1344]),
                        (ps[:, 1024:1408], ID16, oE[:, 576:960]),
                        (ps[:, 1025:1408], ID16, oS[:, 768:1151]),
                        (ps[:, 1024:1407], ID16, oN[:, 769:1152]),
                        (ps[:, 1024:1408], ID16, r16[:, 768:1152]),
                    ],
                    [   # bank 3: chunks 6,7
                        (ps[:, 1536:1920], ID16, oE[:, 960:1344]),
                        (ps[:, 1536:1728], ID16, oW[:, 1344:1536]),
                        (ps[:, 1728:1920], SHU16, oW[:, 0:192]),
                        (ps[:, 1537:1920], ID16, oS[:, 1152:1535]),
                        (ps[:, 1536:1919], ID16, oN[:, 1153:1536]),
                        (ps[:, 1536:1920], ID16, r16[:, 1152:1536]),
                    ],
                ]
                for b in (1, 0, 2, 3):
                    obk = bank_ops[b]
                    for i, (po, w, rh) in enumerate(obk):
                        nc.tensor.matmul(po, w, rh, start=(i == 0),
                                         stop=(i == len(obk) - 1))
                    # drain PSUM -> q on the ACT engine (pure copy: runoff is
                    # already accumulated in PSUM via the r16 matmul).
                    odst = (q32 if lastit else q16)[:, B * b : B * (b + 1)]
                    nc.scalar.copy(odst, ps[:, 512 * b : 512 * b + 384])

            # ---- gradient on owned rows (compact [p, c*128+j] layout)
            s1 = pool.tile([P, 1024], F32, tag="f0", name="f0")
            nc.scalar.sqrt(s1[:], cond[:])
            s2 = pool.tile([P, 1024], F32, tag="f1", name="f1")
            nc.scalar.sqrt(s2[:], s1[:])
            c125 = pool.tile([P, 1024], F32, tag="f0", name="f0b")
            nc.vector.tensor_mul(c125[:], cond[:], s2[:])
            k0 = pool.tile([P, 1024], F32, tag="f1", name="f1b")
            nc.scalar.activation(k0[:], c125[:], ACTF.Square,
                                 scale=float(FLOW_COEFF))
            vo = lambda t: t.rearrange("p (c j) -> p c j", c=NCH)
            km = pool.tile([P, 1024], F32, tag="f0", name="f0c")
            nc.vector.tensor_mul(vo(km), vo(k0), vs(m, OWN0 + 1, OWN))
            q2 = pool.tile([P, 1024], F32, tag="f1", name="f1c")
            nc.scalar.activation(vo(q2), vq(q32, OWN0, OWN), ACTF.Square)
            g = pool.tile([P, 1024], F32, tag="f2", name="f2")
            nc.vector.tensor_mul(g[:], q2[:], km[:])

            nc.sync.dma_start(out=grad_d[:], in_=g[:])

    nc.finalize()
    return nc


# ------------------------------------------------------------------ host side

def _mats():
    ident = np.eye(P, dtype=np.float32)
    shd = np.zeros((P, P), np.float32)
    shd[np.arange(P - 1), np.arange(1, P)] = 1.0      # out[m] = rhs[m-1]
    shu = np.zeros((P, P), np.float32)
    shu[np.arange(1, P), np.arange(P - 1)] = 1.0      # out[m] = rhs[m+1]
    edn = np.zeros((P, P), np.float32)
    edn[P - 1, 0] = 1.0
    eup = np.zeros((P, P), np.float32)
    eup[0, P - 1] = 1.0
    fixc = np.zeros((P, 2 * P), np.float32)
    fixc[0, :] = 1.0e33
    return np.concatenate([ident, shd, shu, edn, eup, fixc], axis=1)


def _to_dev(slab):
    """[rows, 1024] row-major slab -> [128, 8*rows], col = p*8 + c."""
    rows = slab.shape[0]
    return np.ascontiguousarray(
        slab.reshape(rows, P, NCH).transpose(1, 2, 0)).reshape(P, NCH * rows)


_BUILT = None


def _get_built():
    global _BUILT
    if _BUILT is None:
        _BUILT = build()
    return _BUILT


def _make_in_maps(melt_rate, bedrock_elevation, water_pressure, cell_area,
                  conduit_size, status_at_node):
    grid = lambda a: np.asarray(a).reshape(ROWS, COLS)
    bed = grid(bedrock_elevation).astype(np.float32)
    press = grid(water_pressure).astype(np.float32)
    status = grid(status_at_node).astype(np.int32)
    melt = grid(melt_rate).astype(np.float32)
    area = grid(cell_area).astype(np.float32)
    cond = grid(conduit_size).astype(np.float32)

    gp = 33
    bedp = np.full((ROWS + 2 * gp, COLS), PAD_BED, np.float32)
    bedp[gp:gp + ROWS] = bed
    pressp = np.zeros((ROWS + 2 * gp, COLS), np.float32)
    pressp[gp:gp + ROWS] = press
    statusp = np.ones((ROWS + 2 * gp, COLS), np.int32)
    statusp[gp:gp + ROWS] = status
    gq = 32
    meltp = np.zeros((ROWS + 2 * gq, COLS), np.float32)
    meltp[gq:gq + ROWS] = melt
    areap = np.zeros((ROWS + 2 * gq, COLS), np.float32)
    areap[gq:gq + ROWS] = area

    mats = _mats()
    in_maps = []
    for k in range(N_CORES):
        r0 = k * OWN
        in_maps.append({
            "bed": _to_dev(bedp[r0 : r0 + RS]),
            "press": _to_dev(pressp[r0 : r0 + RS]),
            "status": _to_dev(statusp[r0 : r0 + RS]),
            "melt": _to_dev(meltp[r0 : r0 + RQ]),
            "area": _to_dev(areap[r0 : r0 + RQ]),
            "conduit": _to_dev(cond[r0 : r0 + OWN]),
            "mats": mats,
        })
    return in_maps


def _from_dev(res_maps):
    out = np.empty((ROWS, COLS), np.float32)
    for k in range(N_CORES):
        g = res_maps[k]["grad"].reshape(P, NCH, OWN)    # [p, c, j]
        out[k * OWN : (k + 1) * OWN] = g.transpose(2, 0, 1).reshape(OWN, COLS)
    return out.ravel()


def run(inputs, trace=False, **kwargs):
    nc = _get_built()
    in_maps = _make_in_maps(
        inputs["melt_rate"], inputs["bedrock_elevation"],
        inputs["water_pressure"], inputs["cell_area"],
        inputs["conduit_size"], inputs["status_at_node"])
    res = run_bass_kernel_spmd(nc, in_maps, list(range(N_CORES)),
                               trace=trace, **kwargs)
    return _from_dev(res.results), res


def kernel(**inputs):
    out, _ = run(inputs)
    return out


# revision 5
# speedup vs baseline: 1.6949x; 1.0781x over previous
"""Trainium2 Bass kernel for nn_ConduitHydrology (MFD flow accumulation).

The reference graph is the raster 4-neighbor grid on a 1024x1024 raster, so
all segment_sums are 5-point stencil operations. Strategy:
  - Row-partition across 8 cores: core k owns global rows [128k, 128k+128),
    computing on a 192-row slab (32-row halo each side). 32 Jacobi
    iterations x 1-hop stencil => the halo fully absorbs cross-partition
    influence: zero inter-core communication.
  - On-chip layout (interleaved): column = p*8 + c for partition p, chunk
    c in [0,8); rows packed contiguously per chunk (f = c*192 + r for the
    q-domain, c*194 + r for the phi-domain). Row shifts and 7/8 of column
    shifts are free-dim offsets; only the chunk seam (c=7 <-> c=0 of the
    next partition) needs a partition-shift matmul.
  - Per iteration: 4 wide DVE products (one per PSUM bank block; fractions
    stored field-major in one contiguous fALL tensor so each product is a
    single field-strided op against a broadcast q slice), ~24 bf16 matmuls
    on PE accumulating all shifted inflows PLUS the runoff into fp32 PSUM,
    and 4 ACT-engine copies draining PSUM -> bf16 q (fp32 on the last
    iteration). GpSimd is kept idle: concurrent GpSimd tensor ops stretch
    DVE ops 3-5x (SBUF contention).
The host only pads/slices/relayouts numpy arrays (no arithmetic on host).
"""

import numpy as np

import concourse.bass as bass
import concourse.mybir as mybir
from concourse.bacc import Bacc
from concourse.tile import TileContext
from concourse.bass_utils import run_bass_kernel_spmd

F32 = mybir.dt.float32
F16 = mybir.dt.bfloat16
I32 = mybir.dt.int32
ALU = mybir.AluOpType
ACTF = mybir.ActivationFunctionType

ROWS = COLS = 1024
N_CORES = 8
N_ITERS = 32
P = 128
NCH = 8
RQ = 192          # q-domain rows per slab
RS = 194          # phi-domain rows per slab
FQ = NCH * RQ     # 1536
FS = NCH * RS     # 1552
OWN = 128
OWN0 = 32

RHO_W, GRAV, SEC_PER_A = 1000.0, 9.81, 31556926.0
FLOW_COEFF = 0.0405
PAD_BED = 1.0e30


def build(n_iters=N_ITERS):
    nc = Bacc(None)

    bed_d = nc.declare_dram_parameter("bed", [P, FS], F32, isOutput=False)
    press_d = nc.declare_dram_parameter("press", [P, FS], F32, isOutput=False)
    status_d = nc.declare_dram_parameter("status", [P, FS], I32, isOutput=False)
    melt_d = nc.declare_dram_parameter("melt", [P, FQ], F32, isOutput=False)
    area_d = nc.declare_dram_parameter("area", [P, FQ], F32, isOutput=False)
    cond_d = nc.declare_dram_parameter("conduit", [P, 1024], F32, isOutput=False)
    mats_d = nc.declare_dram_parameter("mats", [P, 896], F32, isOutput=False)
    grad_d = nc.declare_dram_parameter("grad", [P, 1024], F32, isOutput=True)

    # phi-domain / q-domain chunk slices (1D)
    sch = lambda t, c, b, n: t[:, c * RS + b : c * RS + b + n]
    qch = lambda t, c, b, n: t[:, c * RQ + b : c * RQ + b + n]
    # 2D chunked views
    vs = lambda t, b, n: t.rearrange("p (c r) -> p c r", c=NCH)[:, :, b : b + n]
    vq = vs

    # setup PSUM layout: chunk c at f = 256*c
    scf = lambda c: 256 * c

    with TileContext(nc) as tc:
        with (
            tc.tile_pool(name="main", bufs=1) as pool,
            tc.tile_pool(name="ps", bufs=2, space="PSUM") as pspool,
        ):
            def tmp(tag):
                return pool.tile([P, FS], F32, tag=tag, name=tag)

            def psum():
                return pspool.tile([P, 2048], F32, tag="ps", name="ps")

            def emit_group(ops):
                """ops: (out_ap, lhsT, rhs_ap, bank). start=True on the first
                matmul touching each PSUM bank (must cover the bank's used
                region), stop on the last."""
                last = {}
                for i, (o, w, rh, bank) in enumerate(ops):
                    last[bank] = i
                seen = set()
                for i, (o, w, rh, bank) in enumerate(ops):
                    st = bank not in seen
                    seen.add(bank)
                    nc.tensor.matmul(o, w, rh, start=st, stop=(last[bank] == i))

            # ---- constants
            mats = pool.tile([P, 896], F32)
            nc.sync.dma_start(out=mats[:], in_=mats_d[:])
            ID = mats[:, 0:128]
            SHD = mats[:, 128:256]   # out[m] = rhs[m-1]
            SHU = mats[:, 256:384]   # out[m] = rhs[m+1]
            EUP = mats[:, 512:640]   # out[127] = rhs[0]
            FIXC = mats[:, 640:896]  # row 0 = 1e33
            mats16 = pool.tile([P, 384], F16)
            nc.vector.tensor_copy(out=mats16[:], in_=mats[:, 0:384])
            ID16 = mats16[:, 0:128]
            SHD16 = mats16[:, 128:256]
            SHU16 = mats16[:, 256:384]

            # ---- inputs
            bed = tmp("t0")
            press = tmp("t1")
            status = pool.tile([P, FS], I32, tag="t2", name="t2")
            melt = tmp("t3")
            area = tmp("t4")
            cond = pool.tile([P, 1024], F32)
            for t, d, n in ((bed, bed_d, FS), (press, press_d, FS),
                            (status, status_d, FS), (melt, melt_d, FQ),
                            (area, area_d, FQ), (cond, cond_d, 1024)):
                nc.sync.dma_start(out=t[:, 0:n], in_=d[:])

            # ---- runoff (q-domain, fp32)
            r = pool.tile([P, FQ], F32)
            nc.vector.scalar_tensor_tensor(
                out=r[:], in0=melt[:, 0:FQ], scalar=1.0 / SEC_PER_A,
                in1=area[:, 0:FQ], op0=ALU.mult, op1=ALU.mult)

            # ---- potential and core mask (phi-domain)
            phi = tmp("t5")
            nc.vector.scalar_tensor_tensor(
                out=phi[:], in0=bed[:], scalar=RHO_W * GRAV,
                in1=press[:], op0=ALU.mult, op1=ALU.add)
            m = pool.tile([P, FS], F32)
            nc.vector.tensor_scalar(
                out=m[:], in0=status[:], scalar1=0, scalar2=None,
                op0=ALU.is_equal)

            # ---- E-neighbor phi / mask. E neighbor of (p,c): (p,c+1) for
            #      c<7, (p+1, chunk 0) for c=7 (seam); none at (p127,c7).
            def shift_from_east(dst, src, fix=None):
                ps = psum()
                ops = [(ps[:, scf(c) : scf(c) + RS], ID, sch(src, c + 1, 0, RS),
                        c // 2) for c in range(NCH - 1)]
                ops.append((ps[:, scf(7) : scf(7) + RS], SHU, sch(src, 0, 0, RS), 3))
                if fix is not None:
                    ops.append((ps[:, scf(7) : scf(7) + RS], EUP, fix[:, 0:RS], 3))
                emit_group(ops)
                nc.scalar.copy(vs(dst, 0, RS),
                               ps.rearrange("p (c r) -> p c r", c=8)[:, :, 0:RS])

            phiE = tmp("t3")
            shift_from_east(phiE, phi, fix=FIXC)
            mE = tmp("t4")
            shift_from_east(mE, m)

            # ---- directional drops (phi-domain link grids)
            dphiE = tmp("t0")
            nc.vector.tensor_sub(dphiE[:], phi[:], phiE[:])
            dropE = tmp("t1")    # flow col -> col+1, stored at col
            nc.vector.scalar_tensor_tensor(
                out=dropE[:], in0=dphiE[:], scalar=0.0, in1=m[:],
                op0=ALU.max, op1=ALU.mult)
            tw = tmp("t3")
            nc.vector.tensor_scalar(
                out=tw[:], in0=dphiE[:], scalar1=-1.0, scalar2=0.0,
                op0=ALU.mult, op1=ALU.max)
            dropW = pool.tile([P, FS], F32, tag="t2", name="t2f")
            nc.vector.tensor_mul(dropW[:], tw[:], mE[:])

            dphiS = tmp("t4")    # phi[r] - phi[r+1], link at r (per chunk)
            nc.vector.tensor_sub(vs(dphiS, 0, RS - 1), vs(phi, 0, RS - 1),
                                 vs(phi, 1, RS - 1))
            dropS = tmp("t6")    # flow r -> r+1, stored at r
            nc.vector.scalar_tensor_tensor(
                out=vs(dropS, 0, RS - 1), in0=vs(dphiS, 0, RS - 1), scalar=0.0,
                in1=vs(m, 0, RS - 1), op0=ALU.max, op1=ALU.mult)
            tn = tmp("t3")
            nc.vector.tensor_scalar(
                out=vs(tn, 0, RS - 1), in0=vs(dphiS, 0, RS - 1), scalar1=-1.0,
                scalar2=0.0, op0=ALU.mult, op1=ALU.max)
            dropN = tmp("t7")    # flow r+1 -> r, stored at r
            nc.vector.tensor_mul(vs(dropN, 0, RS - 1), vs(tn, 0, RS - 1),
                                 vs(m, 1, RS - 1))

            # ---- outgoing-W drop at its source (q-domain):
            #      dW[p,c] = dropW[(p,c-1)] | dropW[(p-1, c7)]
            psW = psum()
            ops = [(psW[:, scf(c) : scf(c) + RQ], ID, sch(dropW, c - 1, 1, RQ),
                    c // 2) for c in range(1, NCH)]
            ops.append((psW[:, scf(0) : scf(0) + RQ], SHD, sch(dropW, 7, 1, RQ), 0))
            emit_group(ops)
            dW = pool.tile([P, FQ], F32, tag="t3", name="t3w")
            nc.scalar.copy(vq(dW, 0, RQ),
                           psW.rearrange("p (c r) -> p c r", c=8)[:, :, 0:RQ])

            # ---- total outgoing drop (q-domain)
            psT = psum()
            ops = []
            for c in range(NCH):
                o = psT[:, scf(c) : scf(c) + RQ]
                ops += [(o, ID, sch(dropE, c, 1, RQ), c // 2),
                        (o, ID, sch(dropS, c, 1, RQ), c // 2),
                        (o, ID, sch(dropN, c, 0, RQ), c // 2),
                        (o, ID, qch(dW, c, 0, RQ), c // 2)]
            emit_group(ops)
            tds = pool.tile([P, FQ], F32, tag="t0", name="t0t")
            nc.vector.tensor_scalar(
                out=vq(tds, 0, RQ),
                in0=psT.rearrange("p (c r) -> p c r", c=8)[:, :, 0:RQ],
                scalar1=1.0e-30, scalar2=None, op0=ALU.max)
            recip = pool.tile([P, FQ], F32, tag="t4", name="t4r")
            nc.vector.reciprocal_approx_fast(out=recip[:], in_=tds[:])

            # ---- outflow fractions, field-major in one contiguous bf16
            #      tensor: fALL = [fE | fW | fS | fN], each [P, FQ].
            fALL = pool.tile([P, 4 * FQ], F16)
            fE = fALL[:, 0 * FQ : 1 * FQ]
            fW = fALL[:, 1 * FQ : 2 * FQ]
            fS = fALL[:, 2 * FQ : 3 * FQ]
            fN = fALL[:, 3 * FQ : 4 * FQ]
            vf = lambda t: t.rearrange("p (c r) -> p c r", c=NCH)
            nc.vector.tensor_mul(vf(fE), vs(dropE, 1, RQ), vq(recip, 0, RQ))
            nc.vector.tensor_mul(fW[:], dW[:], recip[:])
            nc.vector.tensor_mul(vf(fS), vs(dropS, 1, RQ), vq(recip, 0, RQ))
            nc.vector.tensor_mul(vf(fN), vs(dropN, 0, RQ), vq(recip, 0, RQ))

            # slab-edge outflow rows leave the slab; zero them so the
            # pair-merged row-shift matmuls bleed exact zeros across the
            # chunk boundary inside each PSUM bank.
            nc.vector.memset(vf(fS)[:, :, RQ - 1 : RQ], 0.0)
            nc.vector.memset(vf(fN)[:, :, 0:1], 0.0)

            # ---- gradient coefficient (conduit/mask based, q-independent):
            #      km = (FLOW_COEFF * cond^1.25)^2 * core_mask, owned rows.
            #      Computed before the loop so the tail only squares q.
            s1 = pool.tile([P, 1024], F32, tag="f0", name="f0")
            nc.scalar.sqrt(s1[:], cond[:])
            s2 = pool.tile([P, 1024], F32, tag="f1", name="f1")
            nc.scalar.sqrt(s2[:], s1[:])
            c125 = pool.tile([P, 1024], F32, tag="f2", name="f2a")
            nc.vector.tensor_mul(c125[:], cond[:], s2[:])
            k0 = pool.tile([P, 1024], F32, tag="f0", name="f0b")
            nc.scalar.activation(k0[:], c125[:], ACTF.Square,
                                 scale=float(FLOW_COEFF))
            vo = lambda t: t.rearrange("p (c j) -> p c j", c=NCH)
            km = pool.tile([P, 1024], F32, tag="f1", name="f1b")
            nc.vector.tensor_mul(vo(km), vo(k0), vs(m, OWN0 + 1, OWN))

            # ---- discharge iteration state
            r16 = pool.tile([P, FQ], F16)
            nc.scalar.copy(r16[:], r[:])
            q16 = pool.tile([P, FQ], F16)
            nc.scalar.copy(q16[:], r[:])
            q32 = pool.tile([P, FQ], F32)
            oA = pool.tile([P, 4 * FQ], F16)    # iteration products, ping
            oB = pool.tile([P, 4 * FQ], F16)    # iteration products, pong

            B = 384   # q columns per PSUM bank block
            # pair-strided product views: block b covers q cols
            # [384b, 384b+384) across a pair of fields (E,W) or (S,N).
            def fpair(t, pair, b):
                v = t.rearrange("p (f x) -> p f x", f=4)
                return v[:, 2 * pair : 2 * pair + 2, b * B : (b + 1) * B]

            for it in range(n_iters):
                lastit = it == n_iters - 1
                o = oA if it % 2 == 0 else oB
                oE = o[:, 0 * FQ : 1 * FQ]
                oW = o[:, 1 * FQ : 2 * FQ]
                oS = o[:, 2 * FQ : 3 * FQ]
                oN = o[:, 3 * FQ : 4 * FQ]

                # products: two DVE ops per bank block (E,W pair then S,N
                # pair, 2 fields x 384 each), q slice broadcast across the
                # field dim. Order tuned so each PE bank's inputs land early.
                for b in (0, 1, 3, 2):
                    qb = q16[:, b * B : (b + 1) * B]
                    qbc = qb.unsqueeze(1).broadcast_to([P, 2, B])
                    nc.vector.tensor_mul(fpair(o, 0, b), fpair(fALL, 0, b), qbc)
                    nc.vector.tensor_mul(fpair(o, 1, b), fpair(fALL, 1, b), qbc)

                # per-bank matmuls: the dep-free r16 term opens the bank
                # (start=True covers the whole used region) so PE never
                # stalls at iteration heads; row-shift (S/N) terms close it
                # so the drain only gates on this bank's own product block.
                ps = psum()
                bank_ops = [
                    [   # bank 0: chunks 0,1
                        (ps[:, 0:384], ID16, r16[:, 0:384]),
                        (ps[:, 0:384], ID16, oW[:, 192:576]),
                        (ps[:, 0:192], SHD16, oE[:, 1344:1536]),
                        (ps[:, 192:384], ID16, oE[:, 0:192]),
                        (ps[:, 1:384], ID16, oS[:, 0:383]),
                        (ps[:, 0:383], ID16, oN[:, 1:384]),
                    ],
                    [   # bank 1: chunks 2,3
                        (ps[:, 512:896], ID16, r16[:, 384:768]),
                        (ps[:, 512:896], ID16, oW[:, 576:960]),
                        (ps[:, 512:896], ID16, oE[:, 192:576]),
                        (ps[:, 513:896], ID16, oS[:, 384:767]),
                        (ps[:, 512:895], ID16, oN[:, 385:768]),
                    ],
                    [   # bank 2: chunks 4,5
                        (ps[:, 1024:1408], ID16, r16[:, 768:1152]),
                        (ps[:, 1024:1408], ID16, oW[:, 960:1344]),
                        (ps[:, 1024:1408], ID16, oE[:, 576:960]),
                        (ps[:, 1025:1408], ID16, oS[:, 768:1151]),
                        (ps[:, 1024:1407], ID16, oN[:, 769:1152]),
                    ],
                    [   # bank 3: chunks 6,7
                        (ps[:, 1536:1920], ID16, r16[:, 1152:1536]),
                        (ps[:, 1536:1728], ID16, oW[:, 1344:1536]),
                        (ps[:, 1728:1920], SHU16, oW[:, 0:192]),
                        (ps[:, 1536:1920], ID16, oE[:, 960:1344]),
                        (ps[:, 1537:1920], ID16, oS[:, 1152:1535]),
                        (ps[:, 1536:1919], ID16, oN[:, 1153:1536]),
                    ],
                ]
                for b in (0, 1, 3, 2):
                    obk = bank_ops[b]
                    for i, (po, w, rh) in enumerate(obk):
                        nc.tensor.matmul(po, w, rh, start=(i == 0),
                                         stop=(i == len(obk) - 1))
                    # drain PSUM -> q on the ACT engine (pure copy: runoff is
                    # already accumulated in PSUM via the r16 matmul).
                    odst = (q32 if lastit else q16)[:, B * b : B * (b + 1)]
                    nc.scalar.copy(odst, ps[:, 512 * b : 512 * b + 384])

            # ---- gradient on owned rows (compact [p, c*128+j] layout)
            q2 = pool.tile([P, 1024], F32, tag="f0", name="f0c")
            nc.scalar.activation(vo(q2), vq(q32, OWN0, OWN), ACTF.Square)
            g = pool.tile([P, 1024], F32, tag="f2", name="f2b")
            nc.vector.tensor_mul(g[:], q2[:], km[:])

            nc.sync.dma_start(out=grad_d[:], in_=g[:])

    nc.finalize()
    return nc


# ------------------------------------------------------------------ host side

def _mats():
    ident = np.eye(P, dtype=np.float32)
    shd = np.zeros((P, P), np.float32)
    shd[np.arange(P - 1), np.arange(1, P)] = 1.0      # out[m] = rhs[m-1]
    shu = np.zeros((P, P), np.float32)
    shu[np.arange(1, P), np.arange(P - 1)] = 1.0      # out[m] = rhs[m+1]
    edn = np.zeros((P, P), np.float32)
    edn[P - 1, 0] = 1.0
    eup = np.zeros((P, P), np.float32)
    eup[0, P - 1] = 1.0
    fixc = np.zeros((P, 2 * P), np.float32)
    fixc[0, :] = 1.0e33
    return np.concatenate([ident, shd, shu, edn, eup, fixc], axis=1)


def _to_dev(slab):
    """[rows, 1024] row-major slab -> [128, 8*rows], col = p*8 + c."""
    rows = slab.shape[0]
    return np.ascontiguousarray(
        slab.reshape(rows, P, NCH).transpose(1, 2, 0)).reshape(P, NCH * rows)


_BUILT = None


def _get_built():
    global _BUILT
    if _BUILT is None:
        _BUILT = build()
    return _BUILT


def _make_in_maps(melt_rate, bedrock_elevation, water_pressure, cell_area,
                  conduit_size, status_at_node):
    grid = lambda a: np.asarray(a).reshape(ROWS, COLS)
    bed = grid(bedrock_elevation).astype(np.float32)
    press = grid(water_pressure).astype(np.float32)
    status = grid(status_at_node).astype(np.int32)
    melt = grid(melt_rate).astype(np.float32)
    area = grid(cell_area).astype(np.float32)
    cond = grid(conduit_size).astype(np.float32)

    gp = 33
    bedp = np.full((ROWS + 2 * gp, COLS), PAD_BED, np.float32)
    bedp[gp:gp + ROWS] = bed
    pressp = np.zeros((ROWS + 2 * gp, COLS), np.float32)
    pressp[gp:gp + ROWS] = press
    statusp = np.ones((ROWS + 2 * gp, COLS), np.int32)
    statusp[gp:gp + ROWS] = status
    gq = 32
    meltp = np.zeros((ROWS + 2 * gq, COLS), np.float32)
    meltp[gq:gq + ROWS] = melt
    areap = np.zeros((ROWS + 2 * gq, COLS), np.float32)
    areap[gq:gq + ROWS] = area

    mats = _mats()
    in_maps = []
    for k in range(N_CORES):
        r0 = k * OWN
        in_maps.append({
            "bed": _to_dev(bedp[r0 : r0 + RS]),
            "press": _to_dev(pressp[r0 : r0 + RS]),
            "status": _to_dev(statusp[r0 : r0 + RS]),
            "melt": _to_dev(meltp[r0 : r0 + RQ]),
            "area": _to_dev(areap[r0 : r0 + RQ]),
            "conduit": _to_dev(cond[r0 : r0 + OWN]),
            "mats": mats,
        })
    return in_maps


def _from_dev(res_maps):
    out = np.empty((ROWS, COLS), np.float32)
    for k in range(N_CORES):
        g = res_maps[k]["grad"].reshape(P, NCH, OWN)    # [p, c, j]
        out[k * OWN : (k + 1) * OWN] = g.transpose(2, 0, 1).reshape(OWN, COLS)
    return out.ravel()


def run(inputs, trace=False, **kwargs):
    nc = _get_built()
    in_maps = _make_in_maps(
        inputs["melt_rate"], inputs["bedrock_elevation"],
        inputs["water_pressure"], inputs["cell_area"],
        inputs["conduit_size"], inputs["status_at_node"])
    res = run_bass_kernel_spmd(nc, in_maps, list(range(N_CORES)),
                               trace=trace, **kwargs)
    return _from_dev(res.results), res


def kernel(**inputs):
    out, _ = run(inputs)
    return out


# revision 10
# speedup vs baseline: 1.8868x; 1.1132x over previous
"""Trainium2 Bass kernel for nn_ConduitHydrology (MFD flow accumulation).

The reference graph is the raster 4-neighbor grid on a 1024x1024 raster, so
all segment_sums are 5-point stencil operations. Strategy:
  - Row-partition across 8 cores: core k owns global rows [128k, 128k+128),
    computing on a 192-row slab (32-row halo each side). 32 Jacobi
    iterations x 1-hop stencil => the halo fully absorbs cross-partition
    influence: zero inter-core communication.
  - On-chip layout (interleaved): column = p*8 + c for partition p, chunk
    c in [0,8); rows packed contiguously per chunk (f = c*192 + r for the
    q-domain, c*194 + r for the phi-domain). Row shifts and 7/8 of column
    shifts are free-dim offsets; only the chunk seam (c=7 <-> c=0 of the
    next partition) needs a partition-shift matmul.
  - Per iteration: 4 wide DVE products (one per PSUM bank block; the four
    fraction fields live field-major in one contiguous fALL tensor so each
    product is a single field-strided op against a broadcast q slice),
    ~22 bf16 matmuls on PE accumulating all shifted inflows PLUS the
    runoff into fp32 PSUM (the dep-free runoff term opens each bank), and
    4 ACT-engine copies draining PSUM -> bf16 q (fp32 on the last
    iteration). GpSimd stays idle: concurrent GpSimd tensor ops stretch
    DVE ops 3-5x (SBUF contention).
  - Trapezoid: iteration it only needs rows within 31-it of the owned
    block, so all loop APs use per-iteration row windows [lo, hi) that
    shrink by 2 rows per side every 2 iterations (even-aligned to keep
    the DVE 2x perf mode). Out-of-window rows hold stale garbage that by
    construction never feeds a needed row.
The host only pads/slices/relayouts numpy arrays (no arithmetic on host).
"""

import numpy as np

import concourse.bass as bass
import concourse.mybir as mybir
from concourse.bacc import Bacc
from concourse.tile import TileContext
from concourse.bass_utils import run_bass_kernel_spmd

F32 = mybir.dt.float32
F16 = mybir.dt.bfloat16
I32 = mybir.dt.int32
ALU = mybir.AluOpType
ACTF = mybir.ActivationFunctionType

ROWS = COLS = 1024
N_CORES = 8
N_ITERS = 32
P = 128
NCH = 8
RQ = 192          # q-domain rows per slab
RS = 194          # phi-domain rows per slab
FQ = NCH * RQ     # 1536
FS = NCH * RS     # 1552
OWN = 128
OWN0 = 32

RHO_W, GRAV, SEC_PER_A = 1000.0, 9.81, 31556926.0
FLOW_COEFF = 0.0405
PAD_BED = 1.0e30


def build(n_iters=N_ITERS):
    nc = Bacc(None)

    bed_d = nc.declare_dram_parameter("bed", [P, FS], F32, isOutput=False)
    press_d = nc.declare_dram_parameter("press", [P, FS], F32, isOutput=False)
    status_d = nc.declare_dram_parameter("status", [P, FS], I32, isOutput=False)
    melt_d = nc.declare_dram_parameter("melt", [P, FQ], F32, isOutput=False)
    area_d = nc.declare_dram_parameter("area", [P, FQ], F32, isOutput=False)
    cond_d = nc.declare_dram_parameter("conduit", [P, 1024], F32, isOutput=False)
    mats_d = nc.declare_dram_parameter("mats", [P, 896], F32, isOutput=False)
    grad_d = nc.declare_dram_parameter("grad", [P, 1024], F32, isOutput=True)

    # phi-domain chunk slice / chunked views
    sch = lambda t, c, b, n: t[:, c * RS + b : c * RS + b + n]
    vs = lambda t, b, n: t.rearrange("p (c r) -> p c r", c=NCH)[:, :, b : b + n]
    vq = vs

    with TileContext(nc) as tc:
        with (
            tc.tile_pool(name="main", bufs=1) as pool,
            tc.tile_pool(name="ps", bufs=2, space="PSUM") as pspool,
        ):
            def tmp(tag):
                return pool.tile([P, FS], F32, tag=tag, name=tag)

            def psum():
                return pspool.tile([P, 2048], F32, tag="ps", name="ps")

            # ---- inputs (bed/press first: phi gates the setup chain)
            bed = tmp("t0")
            press = tmp("t1")
            mats = pool.tile([P, 896], F32)
            status = pool.tile([P, FS], I32, tag="t2", name="t2")
            melt = tmp("t3")
            area = tmp("t4")
            cond = pool.tile([P, 1024], F32)
            for t, d, n in ((bed, bed_d, FS), (press, press_d, FS),
                            (mats, mats_d, 896), (status, status_d, FS),
                            (melt, melt_d, FQ), (area, area_d, FQ),
                            (cond, cond_d, 1024)):
                nc.sync.dma_start(out=t[:, 0:n], in_=d[:])

            ID = mats[:, 0:128]
            SHU = mats[:, 256:384]   # out[m] = rhs[m+1]
            EUP = mats[:, 512:640]   # out[127] = rhs[0]
            FIXC = mats[:, 640:896]  # row 0 = 1e33
            mats16 = pool.tile([P, 384], F16)
            nc.vector.tensor_copy(out=mats16[:], in_=mats[:, 0:384])
            ID16 = mats16[:, 0:128]
            SHD16 = mats16[:, 128:256]
            SHU16 = mats16[:, 256:384]

            # ---- potential and core mask (phi-domain)
            phi = tmp("t5")
            nc.vector.scalar_tensor_tensor(
                out=phi[:], in0=bed[:], scalar=RHO_W * GRAV,
                in1=press[:], op0=ALU.mult, op1=ALU.add)
            m = pool.tile([P, FS], F32)
            nc.vector.tensor_scalar(
                out=m[:], in0=status[:], scalar1=0, scalar2=None,
                op0=ALU.is_equal)

            # ---- E-W potential drop. E neighbor of (p,c): (p,c+1) for c<7
            #      (a free-dim offset), (p+1, chunk 0) for c=7 (seam matmul).
            F7 = 7 * RS
            dphiE = tmp("t0")   # phi - phiE, phi-domain
            nc.vector.tensor_sub(dphiE[:, 0:F7], phi[:, 0:F7], phi[:, RS:RS + F7])
            psE = psum()        # chunk-7 seam: phi[(p+1, c0)] (+1e33 at p127)
            nc.tensor.matmul(psE[:, 0:RS], SHU, phi[:, 0:RS], start=True, stop=False)
            nc.tensor.matmul(psE[:, 0:RS], EUP, FIXC[:, 0:RS], start=False, stop=False)
            # seam mask mE[(p,c7)] = m[(p+1, c0)] (zero at p127)
            nc.tensor.matmul(psE[:, 256:256 + RS], SHU, m[:, 0:RS],
                             start=True, stop=True)
            nc.vector.tensor_sub(dphiE[:, F7:FS], phi[:, F7:FS], psE[:, 0:RS])

            # ---- directional drops, computed straight into q-domain
            #      alignment (bf16 out), with the phi-row offsets folded into
            #      the fp32 input APs:
            #      dropE_q[c,r] = max(dphiE[c,r+1],0)*m[c,r+1]
            #      dropW_q[c,r] = max(-dphiE[c,r+1],0)*mE[c,r+1]
            #      dropS_q[c,r] = max(phi[c,r+1]-phi[c,r+2],0)*m[c,r+1]
            #      dropN_q[c,r] = max(phi[c,r+2]-phi[c,r+1],0)*m[c,r+2]
            vsq = lambda t, b: t.rearrange(
                "p (c r) -> p c r", c=NCH)[:, :, b : b + RQ]   # phi-domain view
            vq2 = lambda t: t.rearrange("p (c r) -> p c r", c=NCH)  # q-domain

            dE16 = pool.tile([P, FQ], F16)
            nc.vector.scalar_tensor_tensor(
                out=vq2(dE16), in0=vsq(dphiE, 1), scalar=0.0, in1=vsq(m, 1),
                op0=ALU.max, op1=ALU.mult)
            twq = pool.tile([P, FQ], F32, tag="t3b", name="t3b")
            nc.vector.tensor_scalar(
                out=vq2(twq), in0=vsq(dphiE, 1), scalar1=-1.0, scalar2=0.0,
                op0=ALU.mult, op1=ALU.max)
            # mE view: chunks 0..6 read m chunk c+1; chunk 7 from the seam psum
            dW16 = pool.tile([P, FQ], F16)
            mEv = m[:, RS:].rearrange("p (c r) -> p c r", c=NCH - 1)
            nc.vector.tensor_mul(
                vq2(dW16)[:, 0:7], vq2(twq)[:, 0:7], mEv[:, :, 1 : 1 + RQ])
            nc.vector.tensor_mul(
                dW16[:, 7 * RQ : FQ], twq[:, 7 * RQ : FQ],
                psE[:, 257 : 257 + RQ])

            dphiS = tmp("t6")   # phi[r] - phi[r+1] at phi-row r
            nc.vector.tensor_sub(dphiS[:, 0 : FS - 1], phi[:, 0 : FS - 1],
                                 phi[:, 1:FS])
            dS16 = pool.tile([P, FQ], F16)
            nc.vector.scalar_tensor_tensor(
                out=vq2(dS16), in0=vsq(dphiS, 1), scalar=0.0, in1=vsq(m, 1),
                op0=ALU.max, op1=ALU.mult)
            tnq = pool.tile([P, FQ], F32, tag="t0b", name="t0b")
            nc.vector.tensor_scalar(
                out=vq2(tnq), in0=vsq(dphiS, 0), scalar1=-1.0, scalar2=0.0,
                op0=ALU.mult, op1=ALU.max)
            dN16 = pool.tile([P, FQ], F16)
            nc.vector.tensor_mul(vq2(dN16), vq2(tnq), vsq(m, 1))

            # ---- outgoing-W drop at its source node (q-domain, bf16):
            #      dWs[p,c] = dropW_q[(p,c-1)] | dropW_q[(p-1, c7)]
            B = 384
            psW = psum()
            opsW = [(512 * (c // 2) + (c % 2) * RQ, ID16,
                     dW16[:, (c - 1) * RQ : c * RQ], c // 2)
                    for c in range(1, NCH)]
            opsW.append((0, SHD16, dW16[:, 7 * RQ : FQ], 0))
            lastb = {}
            for i, (off, w, rh, bnk) in enumerate(opsW):
                lastb[bnk] = i
            seen = set()
            for i, (off, w, rh, bnk) in enumerate(opsW):
                st = bnk not in seen
                seen.add(bnk)
                nc.tensor.matmul(psW[:, off : off + RQ], w, rh,
                                 start=st, stop=(lastb[bnk] == i))
            dWs = pool.tile([P, FQ], F16)
            nc.scalar.copy(dWs.rearrange("p (b x) -> p b x", b=4),
                           psW.rearrange("p (b x) -> p b x", b=4)[:, :, 0:B])

            # ---- total outgoing drop -> fractions (bf16 matmuls, fp32 psum)
            psT = psum()
            opsT = []
            for b in range(4):
                o = psT[:, 512 * b : 512 * b + B]
                sl = slice(b * B, (b + 1) * B)
                opsT += [(o, ID16, dE16[:, sl]), (o, ID16, dS16[:, sl]),
                         (o, ID16, dN16[:, sl]), (o, ID16, dWs[:, sl])]
            for i, (o, w, rh) in enumerate(opsT):
                nc.tensor.matmul(o, w, rh, start=(i % 4 == 0),
                                 stop=(i % 4 == 3))
            tds = pool.tile([P, FQ], F32, tag="t6b", name="t6b")
            vT = psT.rearrange("p (b x) -> p b x", b=4)[:, :, 0:B]
            nc.vector.tensor_scalar(
                out=tds.rearrange("p (b x) -> p b x", b=4), in0=vT,
                scalar1=1.0e-30, scalar2=None, op0=ALU.max)
            recip = pool.tile([P, FQ], F32, tag="t5b", name="t5b")
            nc.vector.reciprocal_approx_fast(out=recip[:], in_=tds[:])
            rec16 = pool.tile([P, FQ], F16)
            nc.vector.tensor_copy(out=rec16[:], in_=recip[:])

            # ---- outflow fractions, field-major in one contiguous bf16
            #      tensor: fALL = [fE | fW | fS | fN], each [P, FQ].
            fALL = pool.tile([P, 4 * FQ], F16)
            nc.vector.tensor_mul(fALL[:, 0 * FQ : 1 * FQ], dE16[:], rec16[:])
            nc.vector.tensor_mul(fALL[:, 1 * FQ : 2 * FQ], dWs[:], rec16[:])
            nc.vector.tensor_mul(fALL[:, 2 * FQ : 3 * FQ], dS16[:], rec16[:])
            nc.vector.tensor_mul(fALL[:, 3 * FQ : 4 * FQ], dN16[:], rec16[:])

            # slab-edge outflow rows leave the slab; zero them so the it=0
            # full-width row-shift matmuls bleed exact zeros across the
            # chunk boundary inside each PSUM bank.
            fSv = fALL[:, 2 * FQ : 3 * FQ].rearrange("p (c r) -> p c r", c=NCH)
            fNv = fALL[:, 3 * FQ : 4 * FQ].rearrange("p (c r) -> p c r", c=NCH)
            nc.vector.memset(fSv[:, :, RQ - 1 : RQ], 0.0)
            nc.vector.memset(fNv[:, :, 0:1], 0.0)

            # ---- runoff (q-domain)
            r = pool.tile([P, FQ], F32)
            nc.vector.scalar_tensor_tensor(
                out=r[:], in0=melt[:, 0:FQ], scalar=1.0 / SEC_PER_A,
                in1=area[:, 0:FQ], op0=ALU.mult, op1=ALU.mult)

            # ---- gradient coefficient (conduit/mask based, q-independent):
            #      km = (FLOW_COEFF * cond^1.25)^2 * core_mask, owned rows.
            s1 = pool.tile([P, 1024], F32, tag="f0", name="f0")
            nc.scalar.sqrt(s1[:], cond[:])
            s2 = pool.tile([P, 1024], F32, tag="f1", name="f1")
            nc.scalar.sqrt(s2[:], s1[:])
            c125 = pool.tile([P, 1024], F32, tag="f2", name="f2a")
            nc.vector.tensor_mul(c125[:], cond[:], s2[:])
            k0 = pool.tile([P, 1024], F32, tag="f0", name="f0b")
            nc.scalar.activation(k0[:], c125[:], ACTF.Square,
                                 scale=float(FLOW_COEFF))
            vo = lambda t: t.rearrange("p (c j) -> p c j", c=NCH)
            km = pool.tile([P, 1024], F32, tag="f1", name="f1b")
            nc.vector.tensor_mul(vo(km), vo(k0), vs(m, OWN0 + 1, OWN))

            # ---- discharge iteration state
            r16 = pool.tile([P, FQ], F16)
            nc.scalar.copy(r16[:], r[:])
            q16 = pool.tile([P, FQ], F16)
            nc.scalar.copy(q16[:], r[:])
            q32 = pool.tile([P, FQ], F32)
            oA = pool.tile([P, 4 * FQ], F16)    # iteration products, ping
            oB = pool.tile([P, 4 * FQ], F16)    # iteration products, pong

            # 4-field product view of block b restricted to rows [a, a+n)
            def fblk(t, b, a, n):
                v = t.rearrange("p (f c r) -> p f c r", f=4, c=NCH)
                return v[:, :, 2 * b : 2 * b + 2, a : a + n]

            def qwin(t, b, a, n):      # q-domain [2-chunk block, row window]
                v = t.rearrange("p (c r) -> p c r", c=NCH)
                return v[:, 2 * b : 2 * b + 2, a : a + n]

            def pswin(ps, b, a, n):    # PSUM bank b, per-chunk row window
                v = ps[:, 512 * b : 512 * b + 384].rearrange(
                    "p (c r) -> p c r", c=2)
                return v[:, :, a : a + n]

            for it in range(n_iters):
                lastit = it == n_iters - 1
                o = oA if it % 2 == 0 else oB
                oE = o[:, 0 * FQ : 1 * FQ]
                oW = o[:, 1 * FQ : 2 * FQ]
                oS = o[:, 2 * FQ : 3 * FQ]
                oN = o[:, 3 * FQ : 4 * FQ]

                if it == 0:
                    lo, hi = 0, RQ          # full width
                    plo, phi_ = 0, RQ
                else:
                    lo = (1 + it) & ~1
                    hi = (RQ - it) & ~1
                    plo, phi_ = lo - 2, min(RQ, hi + 2)
                n = hi - lo
                pn = phi_ - plo

                # products: one 4-field op per bank block, q broadcast
                for b in (0, 1, 3, 2):
                    qb = qwin(q16, b, plo, pn)
                    nc.vector.tensor_mul(
                        fblk(o, b, plo, pn), fblk(fALL, b, plo, pn),
                        qb.unsqueeze(1).broadcast_to([P, 4, 2, pn]))

                ps = psum()
                ow = lambda t, c0, a, nn: t.rearrange(
                    "p (c r) -> p c r", c=NCH)[:, c0 : c0 + 2, a : a + nn]
                if it == 0:
                    bank_ops = [
                        [   # bank 0: chunks 0,1
                            (ps[:, 0:384], ID16, r16[:, 0:384]),
                            (ps[:, 0:384], ID16, oW[:, 192:576]),
                            (ps[:, 0:192], SHD16, oE[:, 1344:1536]),
                            (ps[:, 192:384], ID16, oE[:, 0:192]),
                            (ps[:, 1:384], ID16, oS[:, 0:383]),
                            (ps[:, 0:383], ID16, oN[:, 1:384]),
                        ],
                        [   # bank 1: chunks 2,3
                            (ps[:, 512:896], ID16, r16[:, 384:768]),
                            (ps[:, 512:896], ID16, oW[:, 576:960]),
                            (ps[:, 512:896], ID16, oE[:, 192:576]),
                            (ps[:, 513:896], ID16, oS[:, 384:767]),
                            (ps[:, 512:895], ID16, oN[:, 385:768]),
                        ],
                        [   # bank 2: chunks 4,5
                            (ps[:, 1024:1408], ID16, r16[:, 768:1152]),
                            (ps[:, 1024:1408], ID16, oW[:, 960:1344]),
                            (ps[:, 1024:1408], ID16, oE[:, 576:960]),
                            (ps[:, 1025:1408], ID16, oS[:, 768:1151]),
                            (ps[:, 1024:1407], ID16, oN[:, 769:1152]),
                        ],
                        [   # bank 3: chunks 6,7
                            (ps[:, 1536:1920], ID16, r16[:, 1152:1536]),
                            (ps[:, 1536:1728], ID16, oW[:, 1344:1536]),
                            (ps[:, 1728:1920], SHU16, oW[:, 0:192]),
                            (ps[:, 1536:1920], ID16, oE[:, 960:1344]),
                            (ps[:, 1537:1920], ID16, oS[:, 1152:1535]),
                            (ps[:, 1536:1919], ID16, oN[:, 1153:1536]),
                        ],
                    ]
                else:
                    # windowed: every term is a [2-chunk, n-row] (or seam
                    # [1-chunk]) AP; row shifts stay inside each chunk.
                    bank_ops = []
                    for b in range(4):
                        c0 = 2 * b
                        po = pswin(ps, b, lo, n)
                        obk = [
                            (po, ID16, qwin(r16, b, lo, n)),
                            # inflow-from-W: source chunks c0-1, c0
                            None,
                            # inflow-from-E: source chunks c0+1, c0+2
                            None,
                            (po, ID16, ow(oS, c0, lo - 1, n)),
                            (po, ID16, ow(oN, c0, lo + 1, n)),
                        ]
                        # W sources
                        if b == 0:
                            obk[1] = [
                                (pswin(ps, 0, lo, n)[:, 0:1, :], SHD16,
                                 vq2(oE)[:, 7:8, lo : lo + n]),
                                (pswin(ps, 0, lo, n)[:, 1:2, :], ID16,
                                 vq2(oE)[:, 0:1, lo : lo + n]),
                            ]
                        else:
                            obk[1] = [(po, ID16, ow(oE, c0 - 1, lo, n))]
                        if b == 3:
                            obk[2] = [
                                (pswin(ps, 3, lo, n)[:, 0:1, :], ID16,
                                 vq2(oW)[:, 7:8, lo : lo + n]),
                                (pswin(ps, 3, lo, n)[:, 1:2, :], SHU16,
                                 vq2(oW)[:, 0:1, lo : lo + n]),
                            ]
                        else:
                            obk[2] = [(po, ID16, ow(oW, c0 + 1, lo, n))]
                        flat = [obk[0]] + obk[1] + obk[2] + [obk[3], obk[4]]
                        bank_ops.append(flat)

                for b in (0, 1, 3, 2):
                    obk = bank_ops[b]
                    for i, (po, w, rh) in enumerate(obk):
                        nc.tensor.matmul(po, w, rh, start=(i == 0),
                                         stop=(i == len(obk) - 1))
                    # drain PSUM -> q on the ACT engine (pure copy: runoff is
                    # already accumulated in PSUM via the r16 matmul).
                    qdst = q32 if lastit else q16
                    nc.scalar.copy(qwin(qdst, b, lo, n), pswin(ps, b, lo, n))

            # ---- gradient on owned rows (compact [p, c*128+j] layout)
            q2 = pool.tile([P, 1024], F32, tag="f0", name="f0c")
            nc.scalar.activation(vo(q2), vq(q32, OWN0, OWN), ACTF.Square)
            g = pool.tile([P, 1024], F32, tag="f2", name="f2b")
            nc.vector.tensor_mul(g[:], q2[:], km[:])

            nc.sync.dma_start(out=grad_d[:], in_=g[:])

    nc.finalize()
    return nc


# ------------------------------------------------------------------ host side

def _mats():
    ident = np.eye(P, dtype=np.float32)
    shd = np.zeros((P, P), np.float32)
    shd[np.arange(P - 1), np.arange(1, P)] = 1.0      # out[m] = rhs[m-1]
    shu = np.zeros((P, P), np.float32)
    shu[np.arange(1, P), np.arange(P - 1)] = 1.0      # out[m] = rhs[m+1]
    edn = np.zeros((P, P), np.float32)
    edn[P - 1, 0] = 1.0
    eup = np.zeros((P, P), np.float32)
    eup[0, P - 1] = 1.0
    fixc = np.zeros((P, 2 * P), np.float32)
    fixc[0, :] = 1.0e33
    return np.concatenate([ident, shd, shu, edn, eup, fixc], axis=1)


def _to_dev(slab):
    """[rows, 1024] row-major slab -> [128, 8*rows], col = p*8 + c."""
    rows = slab.shape[0]
    return np.ascontiguousarray(
        slab.reshape(rows, P, NCH).transpose(1, 2, 0)).reshape(P, NCH * rows)


_BUILT = None


def _get_built():
    global _BUILT
    if _BUILT is None:
        _BUILT = build()
    return _BUILT


def _make_in_maps(melt_rate, bedrock_elevation, water_pressure, cell_area,
                  conduit_size, status_at_node):
    grid = lambda a: np.asarray(a).reshape(ROWS, COLS)
    bed = grid(bedrock_elevation).astype(np.float32)
    press = grid(water_pressure).astype(np.float32)
    status = grid(status_at_node).astype(np.int32)
    melt = grid(melt_rate).astype(np.float32)
    area = grid(cell_area).astype(np.float32)
    cond = grid(conduit_size).astype(np.float32)

    gp = 33
    bedp = np.full((ROWS + 2 * gp, COLS), PAD_BED, np.float32)
    bedp[gp:gp + ROWS] = bed
    pressp = np.zeros((ROWS + 2 * gp, COLS), np.float32)
    pressp[gp:gp + ROWS] = press
    statusp = np.ones((ROWS + 2 * gp, COLS), np.int32)
    statusp[gp:gp + ROWS] = status
    gq = 32
    meltp = np.zeros((ROWS + 2 * gq, COLS), np.float32)
    meltp[gq:gq + ROWS] = melt
    areap = np.zeros((ROWS + 2 * gq, COLS), np.float32)
    areap[gq:gq + ROWS] = area

    mats = _mats()
    in_maps = []
    for k in range(N_CORES):
        r0 = k * OWN
        in_maps.append({
            "bed": _to_dev(bedp[r0 : r0 + RS]),
            "press": _to_dev(pressp[r0 : r0 + RS]),
            "status": _to_dev(statusp[r0 : r0 + RS]),
            "melt": _to_dev(meltp[r0 : r0 + RQ]),
            "area": _to_dev(areap[r0 : r0 + RQ]),
            "conduit": _to_dev(cond[r0 : r0 + OWN]),
            "mats": mats,
        })
    return in_maps


def _from_dev(res_maps):
    out = np.empty((ROWS, COLS), np.float32)
    for k in range(N_CORES):
        g = res_maps[k]["grad"].reshape(P, NCH, OWN)    # [p, c, j]
        out[k * OWN : (k + 1) * OWN] = g.transpose(2, 0, 1).reshape(OWN, COLS)
    return out.ravel()


def run(inputs, trace=False, **kwargs):
    nc = _get_built()
    in_maps = _make_in_maps(
        inputs["melt_rate"], inputs["bedrock_elevation"],
        inputs["water_pressure"], inputs["cell_area"],
        inputs["conduit_size"], inputs["status_at_node"])
    res = run_bass_kernel_spmd(nc, in_maps, list(range(N_CORES)),
                               trace=trace, **kwargs)
    return _from_dev(res.results), res


def kernel(**inputs):
    out, _ = run(inputs)
    return out


# revision 18
# speedup vs baseline: 1.9089x; 1.0117x over previous
"""Trainium2 Bass kernel for nn_ConduitHydrology (MFD flow accumulation).

The reference graph is the raster 4-neighbor grid on a 1024x1024 raster, so
all segment_sums are 5-point stencil operations. Strategy:
  - Row-partition across 8 cores: core k owns global rows [128k, 128k+128),
    computing on a 192-row slab (32-row halo each side). 32 Jacobi
    iterations x 1-hop stencil => the halo fully absorbs cross-partition
    influence: zero inter-core communication.
  - On-chip layout (interleaved): column = p*8 + c for partition p, chunk
    c in [0,8); rows packed contiguously per chunk (f = c*192 + r for the
    q-domain, c*194 + r for the phi-domain). Row shifts and 7/8 of column
    shifts are free-dim offsets; only the chunk seam (c=7 <-> c=0 of the
    next partition) needs a partition-shift matmul.
  - Per iteration: 4 wide DVE products (one per PSUM bank block; the four
    fraction fields live field-major in one contiguous fALL tensor so each
    product is a single field-strided op against a broadcast q slice),
    ~22 bf16 matmuls on PE accumulating all shifted inflows PLUS the
    runoff into fp32 PSUM (the dep-free runoff term opens each bank), and
    4 ACT-engine copies draining PSUM -> bf16 q (fp32 on the last
    iteration). GpSimd stays idle: concurrent GpSimd tensor ops stretch
    DVE ops 3-5x (SBUF contention).
  - Trapezoid: iteration it only needs rows within 31-it of the owned
    block, so all loop APs use per-iteration row windows [lo, hi) that
    shrink by 2 rows per side every 2 iterations (even-aligned to keep
    the DVE 2x perf mode). Out-of-window rows hold stale garbage that by
    construction never feeds a needed row.
The host only pads/slices/relayouts numpy arrays (no arithmetic on host).
"""

import numpy as np

import concourse.bass as bass
import concourse.mybir as mybir
from concourse.bacc import Bacc
from concourse.tile import TileContext
from concourse.bass_utils import run_bass_kernel_spmd

F32 = mybir.dt.float32
F16 = mybir.dt.bfloat16
I32 = mybir.dt.int32
ALU = mybir.AluOpType
ACTF = mybir.ActivationFunctionType

ROWS = COLS = 1024
N_CORES = 8
N_ITERS = 32
P = 128
NCH = 8
RQ = 192          # q-domain rows per slab
RS = 194          # phi-domain rows per slab
FQ = NCH * RQ     # 1536
FS = NCH * RS     # 1552
OWN = 128
OWN0 = 32

RHO_W, GRAV, SEC_PER_A = 1000.0, 9.81, 31556926.0
FLOW_COEFF = 0.0405
PAD_BED = 1.0e30


def build(n_iters=N_ITERS):
    nc = Bacc(None)

    bed_d = nc.declare_dram_parameter("bed", [P, FS], F32, isOutput=False)
    press_d = nc.declare_dram_parameter("press", [P, FS], F32, isOutput=False)
    status_d = nc.declare_dram_parameter("status", [P, FS], I32, isOutput=False)
    melt_d = nc.declare_dram_parameter("melt", [P, FQ], F32, isOutput=False)
    area_d = nc.declare_dram_parameter("area", [P, FQ], F32, isOutput=False)
    cond_d = nc.declare_dram_parameter("conduit", [P, 1024], F32, isOutput=False)
    mats_d = nc.declare_dram_parameter("mats", [P, 896], F32, isOutput=False)
    grad_d = nc.declare_dram_parameter("grad", [P, 1024], F32, isOutput=True)

    # phi-domain chunk slice / chunked views
    sch = lambda t, c, b, n: t[:, c * RS + b : c * RS + b + n]
    vs = lambda t, b, n: t.rearrange("p (c r) -> p c r", c=NCH)[:, :, b : b + n]
    vq = vs

    with TileContext(nc) as tc:
        with (
            tc.tile_pool(name="main", bufs=1) as pool,
            tc.tile_pool(name="ps", bufs=2, space="PSUM") as pspool,
        ):
            def tmp(tag):
                return pool.tile([P, FS], F32, tag=tag, name=tag)

            def psum():
                return pspool.tile([P, 2048], F32, tag="ps", name="ps")

            # ---- inputs (bed/press first: phi gates the setup chain)
            bed = tmp("t0")
            press = tmp("t1")
            mats = pool.tile([P, 896], F32)
            status = pool.tile([P, FS], I32, tag="t2", name="t2")
            melt = tmp("t3")
            area = tmp("t4")
            cond = pool.tile([P, 1024], F32)
            # spread across engine DMA queues so bed/press (which gate the
            # whole setup chain) land as early as possible
            for eng, t, d, n in ((nc.sync, bed, bed_d, FS),
                                 (nc.scalar, press, press_d, FS),
                                 (nc.gpsimd, mats, mats_d, 896),
                                 (nc.gpsimd, status, status_d, FS),
                                 (nc.sync, melt, melt_d, FQ),
                                 (nc.scalar, area, area_d, FQ),
                                 (nc.gpsimd, cond, cond_d, 1024)):
                eng.dma_start(out=t[:, 0:n], in_=d[:])

            ID = mats[:, 0:128]
            SHU = mats[:, 256:384]   # out[m] = rhs[m+1]
            EUP = mats[:, 512:640]   # out[127] = rhs[0]
            FIXC = mats[:, 640:896]  # row 0 = 1e33
            mats16 = pool.tile([P, 384], F16)
            nc.vector.tensor_copy(out=mats16[:], in_=mats[:, 0:384])
            ID16 = mats16[:, 0:128]
            SHD16 = mats16[:, 128:256]
            SHU16 = mats16[:, 256:384]

            # ---- potential and core mask (phi-domain)
            phi = tmp("t5")
            nc.vector.scalar_tensor_tensor(
                out=phi[:], in0=bed[:], scalar=RHO_W * GRAV,
                in1=press[:], op0=ALU.mult, op1=ALU.add)
            m = pool.tile([P, FS], F32)
            nc.vector.tensor_scalar(
                out=m[:], in0=status[:], scalar1=0, scalar2=None,
                op0=ALU.is_equal)

            # ---- E-W potential drop. E neighbor of (p,c): (p,c+1) for c<7
            #      (a free-dim offset), (p+1, chunk 0) for c=7 (seam matmul).
            F7 = 7 * RS
            dphiE = tmp("t0")   # phi - phiE, phi-domain
            nc.vector.tensor_sub(dphiE[:, 0:F7], phi[:, 0:F7], phi[:, RS:RS + F7])
            psE = psum()        # chunk-7 seam: phi[(p+1, c0)] (+1e33 at p127)
            nc.tensor.matmul(psE[:, 0:RS], SHU, phi[:, 0:RS], start=True, stop=False)
            nc.tensor.matmul(psE[:, 0:RS], EUP, FIXC[:, 0:RS], start=False, stop=False)
            # seam mask mE[(p,c7)] = m[(p+1, c0)] (zero at p127)
            nc.tensor.matmul(psE[:, 256:256 + RS], SHU, m[:, 0:RS],
                             start=True, stop=True)
            nc.vector.tensor_sub(dphiE[:, F7:FS], phi[:, F7:FS], psE[:, 0:RS])

            # ---- directional drops, computed straight into q-domain
            #      alignment (bf16 out), with the phi-row offsets folded into
            #      the fp32 input APs:
            #      dropE_q[c,r] = max(dphiE[c,r+1],0)*m[c,r+1]
            #      dropW_q[c,r] = max(-dphiE[c,r+1],0)*mE[c,r+1]
            #      dropS_q[c,r] = max(phi[c,r+1]-phi[c,r+2],0)*m[c,r+1]
            #      dropN_q[c,r] = max(phi[c,r+2]-phi[c,r+1],0)*m[c,r+2]
            vsq = lambda t, b: t.rearrange(
                "p (c r) -> p c r", c=NCH)[:, :, b : b + RQ]   # phi-domain view
            vq2 = lambda t: t.rearrange("p (c r) -> p c r", c=NCH)  # q-domain

            dE16 = pool.tile([P, FQ], F16)
            nc.vector.scalar_tensor_tensor(
                out=vq2(dE16), in0=vsq(dphiE, 1), scalar=0.0, in1=vsq(m, 1),
                op0=ALU.max, op1=ALU.mult)
            twq = pool.tile([P, FQ], F32, tag="t3b", name="t3b")
            nc.vector.tensor_scalar(
                out=vq2(twq), in0=vsq(dphiE, 1), scalar1=-1.0, scalar2=0.0,
                op0=ALU.mult, op1=ALU.max)
            # mE view: chunks 0..6 read m chunk c+1; chunk 7 from the seam psum
            dW16 = pool.tile([P, FQ], F16)
            mEv = m[:, RS:].rearrange("p (c r) -> p c r", c=NCH - 1)
            nc.vector.tensor_mul(
                vq2(dW16)[:, 0:7], vq2(twq)[:, 0:7], mEv[:, :, 1 : 1 + RQ])
            nc.vector.tensor_mul(
                dW16[:, 7 * RQ : FQ], twq[:, 7 * RQ : FQ],
                psE[:, 257 : 257 + RQ])

            dphiS = tmp("t6")   # phi[r] - phi[r+1] at phi-row r
            nc.vector.tensor_sub(dphiS[:, 0 : FS - 1], phi[:, 0 : FS - 1],
                                 phi[:, 1:FS])
            dS16 = pool.tile([P, FQ], F16)
            nc.vector.scalar_tensor_tensor(
                out=vq2(dS16), in0=vsq(dphiS, 1), scalar=0.0, in1=vsq(m, 1),
                op0=ALU.max, op1=ALU.mult)
            tnq = pool.tile([P, FQ], F32, tag="t0b", name="t0b")
            nc.vector.tensor_scalar(
                out=vq2(tnq), in0=vsq(dphiS, 0), scalar1=-1.0, scalar2=0.0,
                op0=ALU.mult, op1=ALU.max)
            dN16 = pool.tile([P, FQ], F16)
            nc.vector.tensor_mul(vq2(dN16), vq2(tnq), vsq(m, 1))

            # ---- outgoing-W drop at its source node (q-domain, bf16):
            #      dWs[p,c] = dropW_q[(p,c-1)] | dropW_q[(p-1, c7)]
            B = 384
            psW = psum()
            opsW = [(512 * (c // 2) + (c % 2) * RQ, ID16,
                     dW16[:, (c - 1) * RQ : c * RQ], c // 2)
                    for c in range(1, NCH)]
            opsW.append((0, SHD16, dW16[:, 7 * RQ : FQ], 0))
            lastb = {}
            for i, (off, w, rh, bnk) in enumerate(opsW):
                lastb[bnk] = i
            seen = set()
            for i, (off, w, rh, bnk) in enumerate(opsW):
                st = bnk not in seen
                seen.add(bnk)
                nc.tensor.matmul(psW[:, off : off + RQ], w, rh,
                                 start=st, stop=(lastb[bnk] == i))
            dWs = pool.tile([P, FQ], F16)
            nc.scalar.copy(dWs.rearrange("p (b x) -> p b x", b=4),
                           psW.rearrange("p (b x) -> p b x", b=4)[:, :, 0:B])

            # ---- total outgoing drop -> fractions (bf16 matmuls, fp32 psum).
            #      A constant 1e-30 term folds the divide-by-zero guard into
            #      the accumulation so the reciprocal reads PSUM directly.
            eps = pool.tile([P, B], F16)
            nc.gpsimd.memset(eps[:], 1.0e-30)
            psT = psum()
            opsT = []
            for b in range(4):
                o = psT[:, 512 * b : 512 * b + B]
                sl = slice(b * B, (b + 1) * B)
                opsT += [(o, ID16, eps[:]), (o, ID16, dE16[:, sl]),
                         (o, ID16, dS16[:, sl]), (o, ID16, dN16[:, sl]),
                         (o, ID16, dWs[:, sl])]
            for i, (o, w, rh) in enumerate(opsT):
                nc.tensor.matmul(o, w, rh, start=(i % 5 == 0),
                                 stop=(i % 5 == 4))
            recip = pool.tile([P, FQ], F32, tag="t5b", name="t5b")
            vT = psT.rearrange("p (b x) -> p b x", b=4)[:, :, 0:B]
            nc.vector.reciprocal_approx_fast(
                out=recip.rearrange("p (b x) -> p b x", b=4), in_=vT)
            rec16 = pool.tile([P, FQ], F16)
            nc.vector.tensor_copy(out=rec16[:], in_=recip[:])

            # ---- outflow fractions, field-major in one contiguous bf16
            #      tensor: fALL = [fE | fW | fS | fN], each [P, FQ].
            fALL = pool.tile([P, 4 * FQ], F16)
            nc.vector.tensor_mul(fALL[:, 0 * FQ : 1 * FQ], dE16[:], rec16[:])
            nc.vector.tensor_mul(fALL[:, 1 * FQ : 2 * FQ], dWs[:], rec16[:])
            nc.vector.tensor_mul(fALL[:, 2 * FQ : 3 * FQ], dS16[:], rec16[:])
            nc.vector.tensor_mul(fALL[:, 3 * FQ : 4 * FQ], dN16[:], rec16[:])

            # slab-edge outflow rows leave the slab; zero them so the it=0
            # full-width row-shift matmuls bleed exact zeros across the
            # chunk boundary inside each PSUM bank.
            fSv = fALL[:, 2 * FQ : 3 * FQ].rearrange("p (c r) -> p c r", c=NCH)
            fNv = fALL[:, 3 * FQ : 4 * FQ].rearrange("p (c r) -> p c r", c=NCH)
            nc.vector.memset(fSv[:, :, RQ - 1 : RQ], 0.0)
            nc.vector.memset(fNv[:, :, 0:1], 0.0)

            # ---- runoff (q-domain)
            r = pool.tile([P, FQ], F32)
            nc.vector.scalar_tensor_tensor(
                out=r[:], in0=melt[:, 0:FQ], scalar=1.0 / SEC_PER_A,
                in1=area[:, 0:FQ], op0=ALU.mult, op1=ALU.mult)

            # ---- discharge iteration state
            r16 = pool.tile([P, FQ], F16)
            nc.scalar.copy(r16[:], r[:])
            q16 = pool.tile([P, FQ], F16)
            nc.scalar.copy(q16[:], r[:])
            q32 = pool.tile([P, FQ], F32)
            oA = pool.tile([P, 4 * FQ], F16)    # iteration products, ping
            oB = pool.tile([P, 4 * FQ], F16)    # iteration products, pong

            # 4-field product view of block b restricted to rows [a, a+n)
            def fblk(t, b, a, n):
                v = t.rearrange("p (f c r) -> p f c r", f=4, c=NCH)
                return v[:, :, 2 * b : 2 * b + 2, a : a + n]

            def qwin(t, b, a, n):      # q-domain [2-chunk block, row window]
                v = t.rearrange("p (c r) -> p c r", c=NCH)
                return v[:, 2 * b : 2 * b + 2, a : a + n]

            def pswin(ps, b, a, n):    # PSUM bank b, per-chunk row window
                v = ps[:, 512 * b : 512 * b + 384].rearrange(
                    "p (c r) -> p c r", c=2)
                return v[:, :, a : a + n]

            for it in range(n_iters):
                lastit = it == n_iters - 1
                o = oA if it % 2 == 0 else oB
                oE = o[:, 0 * FQ : 1 * FQ]
                oW = o[:, 1 * FQ : 2 * FQ]
                oS = o[:, 2 * FQ : 3 * FQ]
                oN = o[:, 3 * FQ : 4 * FQ]

                if it == 0:
                    lo, hi = 0, RQ          # full width
                    plo, phi_ = 0, RQ
                else:
                    lo = (1 + it) & ~1
                    hi = (RQ - it) & ~1
                    plo, phi_ = lo - 2, min(RQ, hi + 2)
                n = hi - lo
                pn = phi_ - plo

                # products: one 4-field op per bank block, q broadcast.
                # Orders rotate by one bank per iteration (barber-pole):
                # bank b's drain lands ~T/4 after bank b-1's, so next
                # iteration's products are emitted in drain-completion order.
                rot = it % 4
                for i in range(4):
                    b = (rot + i) % 4
                    qb = qwin(q16, b, plo, pn)
                    nc.vector.tensor_mul(
                        fblk(o, b, plo, pn), fblk(fALL, b, plo, pn),
                        qb.unsqueeze(1).broadcast_to([P, 4, 2, pn]))

                ps = psum()
                ow = lambda t, c0, a, nn: t.rearrange(
                    "p (c r) -> p c r", c=NCH)[:, c0 : c0 + 2, a : a + nn]
                if it == 0:
                    bank_ops = [
                        [   # bank 0: chunks 0,1
                            (ps[:, 0:384], ID16, r16[:, 0:384]),
                            (ps[:, 0:384], ID16, oW[:, 192:576]),
                            (ps[:, 0:192], SHD16, oE[:, 1344:1536]),
                            (ps[:, 192:384], ID16, oE[:, 0:192]),
                            (ps[:, 1:384], ID16, oS[:, 0:383]),
                            (ps[:, 0:383], ID16, oN[:, 1:384]),
                        ],
                        [   # bank 1: chunks 2,3
                            (ps[:, 512:896], ID16, r16[:, 384:768]),
                            (ps[:, 512:896], ID16, oW[:, 576:960]),
                            (ps[:, 512:896], ID16, oE[:, 192:576]),
                            (ps[:, 513:896], ID16, oS[:, 384:767]),
                            (ps[:, 512:895], ID16, oN[:, 385:768]),
                        ],
                        [   # bank 2: chunks 4,5
                            (ps[:, 1024:1408], ID16, r16[:, 768:1152]),
                            (ps[:, 1024:1408], ID16, oW[:, 960:1344]),
                            (ps[:, 1024:1408], ID16, oE[:, 576:960]),
                            (ps[:, 1025:1408], ID16, oS[:, 768:1151]),
                            (ps[:, 1024:1407], ID16, oN[:, 769:1152]),
                        ],
                        [   # bank 3: chunks 6,7
                            (ps[:, 1536:1920], ID16, r16[:, 1152:1536]),
                            (ps[:, 1536:1728], ID16, oW[:, 1344:1536]),
                            (ps[:, 1728:1920], SHU16, oW[:, 0:192]),
                            (ps[:, 1536:1920], ID16, oE[:, 960:1344]),
                            (ps[:, 1537:1920], ID16, oS[:, 1152:1535]),
                            (ps[:, 1536:1919], ID16, oN[:, 1153:1536]),
                        ],
                    ]
                else:
                    # windowed: every term is a [2-chunk, n-row] (or seam
                    # [1-chunk]) AP; row shifts stay inside each chunk.
                    bank_ops = []
                    for b in range(4):
                        c0 = 2 * b
                        po = pswin(ps, b, lo, n)
                        obk = [
                            (po, ID16, qwin(r16, b, lo, n)),
                            # inflow-from-W: source chunks c0-1, c0
                            None,
                            # inflow-from-E: source chunks c0+1, c0+2
                            None,
                            (po, ID16, ow(oS, c0, lo - 1, n)),
                            (po, ID16, ow(oN, c0, lo + 1, n)),
                        ]
                        # W sources
                        if b == 0:
                            obk[1] = [
                                (pswin(ps, 0, lo, n)[:, 0:1, :], SHD16,
                                 vq2(oE)[:, 7:8, lo : lo + n]),
                                (pswin(ps, 0, lo, n)[:, 1:2, :], ID16,
                                 vq2(oE)[:, 0:1, lo : lo + n]),
                            ]
                        else:
                            obk[1] = [(po, ID16, ow(oE, c0 - 1, lo, n))]
                        if b == 3:
                            obk[2] = [
                                (pswin(ps, 3, lo, n)[:, 0:1, :], ID16,
                                 vq2(oW)[:, 7:8, lo : lo + n]),
                                (pswin(ps, 3, lo, n)[:, 1:2, :], SHU16,
                                 vq2(oW)[:, 0:1, lo : lo + n]),
                            ]
                        else:
                            obk[2] = [(po, ID16, ow(oW, c0 + 1, lo, n))]
                        flat = [obk[0]] + obk[1] + obk[2] + [obk[3], obk[4]]
                        bank_ops.append(flat)

                for i in range(4):
                    b = (rot + 1 + i) % 4
                    obk = bank_ops[b]
                    for j, (po, w, rh) in enumerate(obk):
                        nc.tensor.matmul(po, w, rh, start=(j == 0),
                                         stop=(j == len(obk) - 1))
                    # drain PSUM -> q on the ACT engine (pure copy: runoff is
                    # already accumulated in PSUM via the r16 matmul).
                    qdst = q32 if lastit else q16
                    nc.scalar.copy(qwin(qdst, b, lo, n), pswin(ps, b, lo, n))

                if it == 0:
                    # gradient coefficient km = (FLOW_COEFF*cond^1.25)^2 * m
                    # (q-independent) — emitted here so it runs inside the
                    # early loop iterations' engine bubbles.
                    s1 = pool.tile([P, 1024], F32, tag="f0", name="f0")
                    nc.scalar.sqrt(s1[:], cond[:])
                    s2 = pool.tile([P, 1024], F32, tag="f1", name="f1")
                    nc.scalar.sqrt(s2[:], s1[:])
                    c125 = pool.tile([P, 1024], F32, tag="f2", name="f2a")
                    nc.vector.tensor_mul(c125[:], cond[:], s2[:])
                    k0 = pool.tile([P, 1024], F32, tag="f0", name="f0b")
                    nc.scalar.activation(k0[:], c125[:], ACTF.Square,
                                         scale=float(FLOW_COEFF))
                    vo = lambda t: t.rearrange("p (c j) -> p c j", c=NCH)
                    km = pool.tile([P, 1024], F32, tag="f1", name="f1b")
                    nc.vector.tensor_mul(vo(km), vo(k0), vs(m, OWN0 + 1, OWN))

            # ---- gradient on owned rows (compact [p, c*128+j] layout),
            #      per-block so squares/muls/output DMAs pipeline behind the
            #      last iteration's drains, DMAs spread over 4 queues.
            q2 = pool.tile([P, 1024], F32, tag="f0", name="f0c")
            g = pool.tile([P, 1024], F32, tag="f2", name="f2b")
            outq = (nc.sync, nc.scalar, nc.gpsimd, nc.sync)
            for b in range(4):
                sl = slice(256 * b, 256 * b + 256)
                nc.scalar.activation(vo(q2)[:, 2 * b : 2 * b + 2, :],
                                     vq(q32, OWN0, OWN)[:, 2 * b : 2 * b + 2, :],
                                     ACTF.Square)
                nc.vector.tensor_mul(g[:, sl], q2[:, sl], km[:, sl])
                outq[b].dma_start(out=grad_d[:, sl], in_=g[:, sl])

    nc.finalize()
    return nc


# ------------------------------------------------------------------ host side

def _mats():
    ident = np.eye(P, dtype=np.float32)
    shd = np.zeros((P, P), np.float32)
    shd[np.arange(P - 1), np.arange(1, P)] = 1.0      # out[m] = rhs[m-1]
    shu = np.zeros((P, P), np.float32)
    shu[np.arange(1, P), np.arange(P - 1)] = 1.0      # out[m] = rhs[m+1]
    edn = np.zeros((P, P), np.float32)
    edn[P - 1, 0] = 1.0
    eup = np.zeros((P, P), np.float32)
    eup[0, P - 1] = 1.0
    fixc = np.zeros((P, 2 * P), np.float32)
    fixc[0, :] = 1.0e33
    return np.concatenate([ident, shd, shu, edn, eup, fixc], axis=1)


def _to_dev(slab):
    """[rows, 1024] row-major slab -> [128, 8*rows], col = p*8 + c."""
    rows = slab.shape[0]
    return np.ascontiguousarray(
        slab.reshape(rows, P, NCH).transpose(1, 2, 0)).reshape(P, NCH * rows)


_BUILT = None


def _get_built():
    global _BUILT
    if _BUILT is None:
        _BUILT = build()
    return _BUILT


def _make_in_maps(melt_rate, bedrock_elevation, water_pressure, cell_area,
                  conduit_size, status_at_node):
    grid = lambda a: np.asarray(a).reshape(ROWS, COLS)
    bed = grid(bedrock_elevation).astype(np.float32)
    press = grid(water_pressure).astype(np.float32)
    status = grid(status_at_node).astype(np.int32)
    melt = grid(melt_rate).astype(np.float32)
    area = grid(cell_area).astype(np.float32)
    cond = grid(conduit_size).astype(np.float32)

    gp = 33
    bedp = np.full((ROWS + 2 * gp, COLS), PAD_BED, np.float32)
    bedp[gp:gp + ROWS] = bed
    pressp = np.zeros((ROWS + 2 * gp, COLS), np.float32)
    pressp[gp:gp + ROWS] = press
    statusp = np.ones((ROWS + 2 * gp, COLS), np.int32)
    statusp[gp:gp + ROWS] = status
    gq = 32
    meltp = np.zeros((ROWS + 2 * gq, COLS), np.float32)
    meltp[gq:gq + ROWS] = melt
    areap = np.zeros((ROWS + 2 * gq, COLS), np.float32)
    areap[gq:gq + ROWS] = area

    mats = _mats()
    in_maps = []
    for k in range(N_CORES):
        r0 = k * OWN
        in_maps.append({
            "bed": _to_dev(bedp[r0 : r0 + RS]),
            "press": _to_dev(pressp[r0 : r0 + RS]),
            "status": _to_dev(statusp[r0 : r0 + RS]),
            "melt": _to_dev(meltp[r0 : r0 + RQ]),
            "area": _to_dev(areap[r0 : r0 + RQ]),
            "conduit": _to_dev(cond[r0 : r0 + OWN]),
            "mats": mats,
        })
    return in_maps


def _from_dev(res_maps):
    out = np.empty((ROWS, COLS), np.float32)
    for k in range(N_CORES):
        g = res_maps[k]["grad"].reshape(P, NCH, OWN)    # [p, c, j]
        out[k * OWN : (k + 1) * OWN] = g.transpose(2, 0, 1).reshape(OWN, COLS)
    return out.ravel()


def run(inputs, trace=False, **kwargs):
    nc = _get_built()
    in_maps = _make_in_maps(
        inputs["melt_rate"], inputs["bedrock_elevation"],
        inputs["water_pressure"], inputs["cell_area"],
        inputs["conduit_size"], inputs["status_at_node"])
    res = run_bass_kernel_spmd(nc, in_maps, list(range(N_CORES)),
                               trace=trace, **kwargs)
    return _from_dev(res.results), res


def kernel(**inputs):
    out, _ = run(inputs)
    return out
